# revision 6
# baseline (speedup 1.0000x reference)
"""Trainium2 Bass kernel for nn_MAMoE (conv-MoE -> row attention -> MLP-MoE).

Sharding: 8 cores = (batch b in 0..3) x (H-half in 0..1). All routing is
per-token; the reference's swapaxes(1,2) means attention row r produces
output column w=r, so each core independently computes the full pipeline
for its 48 attention rows and the host reassembles along W.

All large matmuls run as float32r (full-rate fp32 storage, ~1e-3 matmul
accuracy); small-N attention matmuls run fp32.
"""
import numpy as np

import concourse.bass as bass
import concourse.mybir as mybir
import concourse.tile as tile
from concourse import bacc
from concourse.bass_utils import run_bass_kernel_spmd
from concourse.masks import make_identity

F32 = mybir.dt.float32
F32R = mybir.dt.float32r

B, HH, WW, C = 4, 96, 96, 384
HD = 128
SCALE = float((HD // 3) ** -0.5)  # 42**-0.5
N_CORES = 8
R = 48            # attention rows per core
RP = 64           # padded rows in x slab (R + 2*8)
SP = 112          # padded W (96 + 2*8)
T = R * 96        # tokens per core = 4608
NT = 512          # tokens per MLP tile
NTILES = T // NT  # 9
GROUPS = R // 4   # 12 groups of 4 rows
GN = 4 * 96       # tokens per group = 384

TAPS_A = [
    [(dr, ds) for dr in (-1, 0, 1) for ds in (-1, 0, 1)],
    [(dr, 0) for dr in range(-4, 5)],
    [(0, ds) for ds in range(-4, 5)],
]
TAPS_B = [
    [(dr, ds) for dr in (-2, 0, 2) for ds in (-2, 0, 2)],
    [(dr, 0) for dr in range(-8, 9, 2)],
    [(0, ds) for ds in range(-8, 9, 2)],
]

_CACHED_NC = None


def build_kernel():
    nc = bacc.Bacc("TRN2", target_bir_lowering=False, debug=False)

    xp = nc.dram_tensor("xp", [C, RP, SP], F32R, kind="ExternalInput").ap()
    wca = nc.dram_tensor("wca", [3, 9, HD, HD], F32R, kind="ExternalInput").ap()
    wcb = nc.dram_tensor("wcb", [3, 9, HD, HD], F32R, kind="ExternalInput").ap()
    bca = nc.dram_tensor("bca", [HD, 3], F32, kind="ExternalInput").ap()
    bcb = nc.dram_tensor("bcb", [HD, 3], F32, kind="ExternalInput").ap()
    wgd = nc.dram_tensor("wgd", [3, HD, 1], F32R, kind="ExternalInput").ap()
    eb3 = nc.dram_tensor("eb3", [3, 384], F32R, kind="ExternalInput").ap()
    onesd = nc.dram_tensor("onesd", [1, HD], F32R, kind="ExternalInput").ap()
    wqk = nc.dram_tensor("wqk", [3, HD, 256], F32R, kind="ExternalInput").ap()
    wv = nc.dram_tensor("wv", [3, HD, HD], F32R, kind="ExternalInput").ap()
    wap = nc.dram_tensor("wap", [3, HD, HD], F32R, kind="ExternalInput").ap()
    bap = nc.dram_tensor("bap", [HD, 3], F32, kind="ExternalInput").ap()
    wgf = nc.dram_tensor("wgf", [3, HD, 3], F32R, kind="ExternalInput").ap()
    w1 = nc.dram_tensor("w1", [3, 3, HD, 1536], F32R, kind="ExternalInput").ap()
    b1 = nc.dram_tensor("b1", [HD, 3, 12], F32, kind="ExternalInput").ap()
    w2 = nc.dram_tensor("w2", [3, 12, HD, C], F32R, kind="ExternalInput").ap()
    b2r = nc.dram_tensor("b2r", [3, C], F32R, kind="ExternalInput").ap()
    wpr = nc.dram_tensor("wpr", [3, HD, C], F32R, kind="ExternalInput").ap()
    bpr = nc.dram_tensor("bpr", [HD, 3], F32, kind="ExternalInput").ap()
    out_cm = nc.dram_tensor("out_cm", [C, T], F32, kind="ExternalOutput").ap()

    with tile.TileContext(nc) as tc:
        with tc.tile_pool(name="consts", bufs=1) as consts, \
             tc.tile_pool(name="persist", bufs=1) as persist:
            ones_r = consts.tile([1, HD], F32R)
            nc.sync.dma_start(out=ones_r, in_=onesd)
            ident = consts.tile([HD, HD], F32)
            make_identity(nc, ident)

            bca_sb = persist.tile([HD, 3], F32)
            nc.sync.dma_start(out=bca_sb, in_=bca)
            bcb_sb = persist.tile([HD, 3], F32)
            nc.sync.dma_start(out=bcb_sb, in_=bcb)
            bap_sb = persist.tile([HD, 3], F32)
            nc.sync.dma_start(out=bap_sb, in_=bap)

            xc_t = [persist.tile([HD, T], F32R, tag=f"xc{i}", name=f"xc{i}") for i in range(3)]

            # ---------------- Phase A: conv MoE + attention per branch ----
            with tc.tile_pool(name="xpool", bufs=2) as xpool, \
                 tc.tile_pool(name="wpoolA", bufs=2) as wpoolA, \
                 tc.tile_pool(name="gpool", bufs=2) as gpool, \
                 tc.tile_pool(name="psA", bufs=8, space="PSUM") as psA:
                for i in range(3):
                    xp_sb = xpool.tile([HD, RP, SP], F32R, tag="xp")
                    nc.sync.dma_start(out=xp_sb, in_=xp[i * HD:(i + 1) * HD])
                    wca_sb = wpoolA.tile([HD, 9, HD], F32R, tag="wca")
                    nc.sync.dma_start(out=wca_sb, in_=wca[i].rearrange("a p b -> p a b"))
                    wcb_sb = wpoolA.tile([HD, 9, HD], F32R, tag="wcb")
                    nc.sync.dma_start(out=wcb_sb, in_=wcb[i].rearrange("a p b -> p a b"))
                    wgd_sb = wpoolA.tile([HD, 1], F32R, tag="wgd")
                    nc.sync.dma_start(out=wgd_sb, in_=wgd[i])
                    wqk_sb = wpoolA.tile([HD, 256], F32R, tag="wqk")
                    nc.sync.dma_start(out=wqk_sb, in_=wqk[i])
                    wv_sb = wpoolA.tile([HD, HD], F32R, tag="wv")
                    nc.sync.dma_start(out=wv_sb, in_=wv[i])
                    wap_sb = wpoolA.tile([HD, HD], F32R, tag="wap")
                    nc.sync.dma_start(out=wap_sb, in_=wap[i])

                    for g in range(GROUPS):
                        rb = 8 + 4 * g  # xp row of first moe row in group
                        # two expert convs, 9 taps each, accumulated in PSUM
                        pa = psA.tile([HD, GN], F32, tag="ps")
                        for ti, (dr, ds) in enumerate(TAPS_A[i]):
                            nc.tensor.matmul(
                                pa, wca_sb[:, ti, :],
                                xp_sb[:, rb + dr: rb + dr + 4, 8 + ds: 104 + ds],
                                start=(ti == 0), stop=(ti == 8))
                        pb = psA.tile([HD, GN], F32, tag="ps")
                        for ti, (dr, ds) in enumerate(TAPS_B[i]):
                            nc.tensor.matmul(
                                pb, wcb_sb[:, ti, :],
                                xp_sb[:, rb + dr: rb + dr + 4, 8 + ds: 104 + ds],
                                start=(ti == 0), stop=(ti == 8))
                        # 2-expert gate: g0 = sigmoid(-(l1 - l0))
                        plg = psA.tile([1, GN], F32, tag="ps")
                        nc.tensor.matmul(plg, wgd_sb,
                                         xp_sb[:, rb: rb + 4, 8:104],
                                         start=True, stop=True)
                        g0 = gpool.tile([1, GN], F32R, tag="g0")
                        nc.scalar.activation(g0, plg,
                                             mybir.ActivationFunctionType.Sigmoid,
                                             scale=-1.0)
                        pgb = psA.tile([HD, GN], F32, tag="ps")
                        nc.tensor.matmul(pgb, ones_r, g0, start=True, stop=True)
                        # moe = g0*(ca - cb) + cb  (bias-add fused on ACT)
                        ca = gpool.tile([HD, GN], F32, tag="ca")
                        nc.scalar.activation(ca, pa,
                                             mybir.ActivationFunctionType.Identity,
                                             bias=bca_sb[:, i:i + 1])
                        cb = gpool.tile([HD, GN], F32, tag="cb")
                        nc.scalar.activation(cb, pb,
                                             mybir.ActivationFunctionType.Identity,
                                             bias=bcb_sb[:, i:i + 1])
                        dd = gpool.tile([HD, GN], F32, tag="dd")
                        nc.vector.tensor_sub(dd, ca, cb)
                        d2 = gpool.tile([HD, GN], F32, tag="d2")
                        nc.vector.tensor_mul(d2, dd, pgb)
                        moe = gpool.tile([HD, GN], F32R, tag="moe")
                        nc.vector.tensor_add(moe, d2, cb)
                        # q, k projections (channel-major)
                        pq = psA.tile([HD, GN], F32, tag="ps")
                        nc.tensor.matmul(pq, wqk_sb[:, 0:HD], moe,
                                         start=True, stop=True)
                        q_sb = gpool.tile([HD, GN], F32, tag="q")
                        nc.vector.tensor_copy(q_sb, pq)
                        pk = psA.tile([HD, GN], F32, tag="ps")
                        nc.tensor.matmul(pk, wqk_sb[:, HD:256], moe,
                                         start=True, stop=True)
                        k_sb = gpool.tile([HD, GN], F32, tag="k")
                        nc.vector.tensor_copy(k_sb, pk)
                        # vT per row: [96 tok, 128 ch]
                        pvt = psA.tile([96, 4 * HD], F32, tag="ps")
                        for j in range(4):
                            nc.tensor.matmul(pvt[:, j * HD:(j + 1) * HD],
                                             moe[:, j * 96:(j + 1) * 96], wv_sb,
                                             start=True, stop=True)
                        vt_sb = gpool.tile([96, 4 * HD], F32, tag="vt")
                        nc.vector.tensor_copy(vt_sb, pvt)
                        # scores + softmax (no max-sub: logits are tiny)
                        psc = psA.tile([96, GN], F32, tag="ps")
                        for j in range(4):
                            nc.tensor.matmul(psc[:, j * 96:(j + 1) * 96],
                                             q_sb[:, j * 96:(j + 1) * 96],
                                             k_sb[:, j * 96:(j + 1) * 96],
                                             start=True, stop=True)
                        probs = gpool.tile([96, GN], F32, tag="probs")
                        nc.scalar.activation(probs, psc,
                                             mybir.ActivationFunctionType.Exp,
                                             scale=SCALE)
                        zsum = gpool.tile([96, 4], F32, tag="zsum")
                        nc.vector.tensor_reduce(
                            zsum, probs.rearrange("p (j q) -> p j q", q=96),
                            axis=mybir.AxisListType.X, op=mybir.AluOpType.add)
                        rec = gpool.tile([96, 4], F32, tag="rec")
                        nc.vector.reciprocal(rec, zsum)
                        pn = gpool.tile([96, GN], F32, tag="pn")
                        for j in range(4):
                            nc.vector.tensor_scalar_mul(
                                pn[:, j * 96:(j + 1) * 96],
                                probs[:, j * 96:(j + 1) * 96],
                                rec[:, j:j + 1])
                        ppt = psA.tile([96, GN], F32, tag="ps")
                        for j in range(4):
                            nc.tensor.transpose(ppt[:, j * 96:(j + 1) * 96],
                                                pn[:, j * 96:(j + 1) * 96],
                                                ident[:96, :96])
                        pt_sb = gpool.tile([96, GN], F32, tag="pt")
                        nc.vector.tensor_copy(pt_sb, ppt)
                        po = psA.tile([HD, GN], F32, tag="ps")
                        for j in range(4):
                            nc.tensor.matmul(po[:, j * 96:(j + 1) * 96],
                                             vt_sb[:, j * HD:(j + 1) * HD],
                                             pt_sb[:, j * 96:(j + 1) * 96],
                                             start=True, stop=True)
                        og = gpool.tile([HD, GN], F32R, tag="og")
                        nc.vector.tensor_copy(og, po)
                        pap2 = psA.tile([HD, GN], F32, tag="ps")
                        nc.tensor.matmul(pap2, wap_sb, og, start=True, stop=True)
                        nc.scalar.activation(
                            xc_t[i][:, g * GN:(g + 1) * GN], pap2,
                            mybir.ActivationFunctionType.Identity,
                            bias=bap_sb[:, i:i + 1])

            # ---------------- Phase B: final MLP MoE + proj ---------------
            with tc.tile_pool(name="wpoolB", bufs=1) as wpoolB, \
                 tc.tile_pool(name="bpool", bufs=2) as bpool, \
                 tc.tile_pool(name="spool", bufs=1) as spool, \
                 tc.tile_pool(name="psL", bufs=3, space="PSUM") as psL, \
                 tc.tile_pool(name="psB", bufs=5, space="PSUM") as psB:
                w1_sb = []
                w2_sb = []
                for e in range(3):
                    t1 = wpoolB.tile([HD, 3, 1536], F32R, tag=f"w1_{e}", name=f"w1_{e}")
                    nc.sync.dma_start(out=t1, in_=w1[e].rearrange("a p b -> p a b"))
                    w1_sb.append(t1)
                    t2 = wpoolB.tile([HD, 12, C], F32R, tag=f"w2_{e}", name=f"w2_{e}")
                    nc.sync.dma_start(out=t2, in_=w2[e].rearrange("a p b -> p a b"))
                    w2_sb.append(t2)
                b1_sb = wpoolB.tile([HD, 3, 12], F32)
                nc.sync.dma_start(out=b1_sb, in_=b1)
                b2r_sb = wpoolB.tile([3, C], F32R)
                nc.sync.dma_start(out=b2r_sb, in_=b2r)
                wgf_sb = wpoolB.tile([HD, 3, 3], F32R)
                nc.sync.dma_start(out=wgf_sb, in_=wgf.rearrange("a p b -> p a b"))
                wpr_sb = wpoolB.tile([HD, 3, C], F32R)
                nc.sync.dma_start(out=wpr_sb, in_=wpr.rearrange("a p b -> p a b"))
                bpr_sb = wpoolB.tile([HD, 3], F32)
                nc.sync.dma_start(out=bpr_sb, in_=bpr)
                eb3_sb = wpoolB.tile([3, 384], F32R)
                nc.sync.dma_start(out=eb3_sb, in_=eb3)

                for t in range(NTILES):
                    t0 = t * NT
                    # gating logits -> [3, NT] psum
                    plg = psB.tile([3, NT], F32, tag="ps")
                    for kc in range(3):
                        nc.tensor.matmul(plg, wgf_sb[:, kc, :],
                                         xc_t[kc][:, t0:t0 + NT],
                                         start=(kc == 0), stop=(kc == 2))
                    lsb = spool.tile([3, NT], F32, tag="lsb")
                    nc.vector.tensor_copy(lsb, plg)
                    # token-major logits [128, 4, 3]
                    plt = psB.tile([HD, 12], F32, tag="ps")
                    for t4 in range(4):
                        nc.tensor.transpose(plt[:, t4 * 3:(t4 + 1) * 3],
                                            lsb[:, t4 * HD:(t4 + 1) * HD],
                                            ident[:3, :3])
                    lt = spool.tile([HD, 12], F32, tag="lt")
                    nc.vector.tensor_copy(lt, plt)
                    e_sb = spool.tile([HD, 12], F32, tag="e_sb")
                    nc.scalar.activation(e_sb, lt,
                                         mybir.ActivationFunctionType.Exp)
                    e3 = e_sb.rearrange("p (j e) -> p j e", e=3)
                    s4 = spool.tile([HD, 4], F32, tag="s4")
                    nc.vector.tensor_reduce(s4, e3, axis=mybir.AxisListType.X,
                                            op=mybir.AluOpType.add)
                    mn = spool.tile([HD, 4], F32, tag="mn")
                    nc.vector.tensor_reduce(mn, e3, axis=mybir.AxisListType.X,
                                            op=mybir.AluOpType.min)
                    den = spool.tile([HD, 4], F32, tag="den")
                    nc.vector.tensor_sub(den, s4, mn)
                    rec = spool.tile([HD, 4], F32, tag="recb")
                    nc.vector.reciprocal(rec, den)
                    gfin = spool.tile([HD, 12], F32, tag="gfin")
                    nmask = spool.tile([HD, 12], F32, tag="nmask")
                    for t4 in range(4):
                        sl = slice(t4 * 3, (t4 + 1) * 3)
                        nc.vector.tensor_scalar_mul(gfin[:, sl], e_sb[:, sl],
                                                    rec[:, t4:t4 + 1])
                        nc.vector.tensor_scalar(nmask[:, sl], e_sb[:, sl],
                                                mn[:, t4:t4 + 1], None,
                                                op0=mybir.AluOpType.not_equal)
                    gm = spool.tile([HD, 12], F32, tag="gm")
                    nc.vector.tensor_mul(gm, gfin, nmask)
                    # back to expert-major [3, NT]
                    pgt = psB.tile([3, NT], F32, tag="ps")
                    for t4 in range(4):
                        nc.tensor.transpose(pgt[:, t4 * HD:(t4 + 1) * HD],
                                            gm[:, t4 * 3:(t4 + 1) * 3],
                                            ident)
                    gates_r = spool.tile([3, NT], F32R, tag="gates")
                    nc.vector.tensor_copy(gates_r, pgt)

                    pd = [psL.tile([HD, NT], F32, tag="down", name=f"pd{_i}") for _i in range(3)]
                    for e in range(3):
                        pgb = psB.tile([HD, NT], F32, tag="ps")
                        nc.tensor.matmul(pgb, eb3_sb[:, e * HD:(e + 1) * HD],
                                         gates_r, start=True, stop=True)
                        for m in range(12):
                            pu = psB.tile([HD, NT], F32, tag="ps")
                            for kc in range(3):
                                nc.tensor.matmul(
                                    pu, w1_sb[e][:, kc, m * HD:(m + 1) * HD],
                                    xc_t[kc][:, t0:t0 + NT],
                                    start=(kc == 0), stop=(kc == 2))
                            h = bpool.tile([HD, NT], F32, tag="h")
                            nc.scalar.activation(
                                h, pu, mybir.ActivationFunctionType.Gelu,
                                bias=b1_sb[:, e, m:m + 1])
                            hs = bpool.tile([HD, NT], F32R, tag="hs")
                            nc.vector.tensor_mul(hs, h, pgb)
                            for mp in range(3):
                                nc.tensor.matmul(
                                    pd[mp], w2_sb[e][:, m, mp * HD:(mp + 1) * HD],
                                    hs, start=(e == 0 and m == 0), stop=False)
                    for mp in range(3):
                        nc.tensor.matmul(pd[mp], b2r_sb[:, mp * HD:(mp + 1) * HD],
                                         gates_r, start=False, stop=True)
                    dsb = []
                    for mp in range(3):
                        dt_ = spool.tile([HD, NT], F32R, tag=f"dsb{mp}", name=f"dsb{mp}")
                        nc.vector.tensor_copy(dt_, pd[mp])
                        dsb.append(dt_)
                    for mp in range(3):
                        ppj = psB.tile([HD, NT], F32, tag="ps")
                        for kc in range(3):
                            nc.tensor.matmul(ppj,
                                             wpr_sb[:, kc, mp * HD:(mp + 1) * HD],
                                             dsb[kc],
                                             start=(kc == 0), stop=(kc == 2))
                        osb = bpool.tile([HD, NT], F32, tag="osb")
                        nc.scalar.activation(osb, ppj,
                                             mybir.ActivationFunctionType.Identity,
                                             bias=bpr_sb[:, mp:mp + 1])
                        nc.sync.dma_start(
                            out=out_cm[mp * HD:(mp + 1) * HD, t0:t0 + NT],
                            in_=osb)
    nc.compile()
    return nc


def _prep_inputs(x, w_e1, b_e1, w_e2, b_e2, w_e3, b_e3, w_e4, b_e4, w_e5, b_e5,
                 w_e6, b_e6, wg1, wg2, wg3, w_qkv, w_attn_proj, b_attn_proj,
                 wg_final, w_mlp1, b_mlp1, w_mlp2, b_mlp2, w_proj, b_proj):
    f = np.float32
    shared = {}
    shared["wca"] = np.ascontiguousarray(np.stack([
        w_e1.reshape(9, HD, HD), w_e3.reshape(9, HD, HD),
        w_e5.reshape(9, HD, HD)]), dtype=f)
    shared["wcb"] = np.ascontiguousarray(np.stack([
        w_e2.reshape(9, HD, HD), w_e4.reshape(9, HD, HD),
        w_e6.reshape(9, HD, HD)]), dtype=f)
    shared["bca"] = np.ascontiguousarray(np.stack([b_e1, b_e3, b_e5], axis=1), dtype=f)
    shared["bcb"] = np.ascontiguousarray(np.stack([b_e2, b_e4, b_e6], axis=1), dtype=f)
    wgs = np.stack([wg1, wg2, wg3])
    shared["wgd"] = np.ascontiguousarray(
        (wgs[:, :, 1] - wgs[:, :, 0])[:, :, None], dtype=f)
    eb3 = np.zeros((3, 384), f)
    for e in range(3):
        eb3[e, e * 128:(e + 1) * 128] = 1.0
    shared["eb3"] = eb3
    shared["onesd"] = np.ones((1, 128), f)
    shared["wqk"] = np.ascontiguousarray(w_qkv[:, :, :256], dtype=f)
    shared["wv"] = np.ascontiguousarray(w_qkv[:, :, 256:], dtype=f)
    shared["wap"] = np.ascontiguousarray(w_attn_proj, dtype=f)
    shared["bap"] = np.ascontiguousarray(b_attn_proj.T, dtype=f)
    shared["wgf"] = np.ascontiguousarray(wg_final.reshape(3, HD, 3), dtype=f)
    shared["w1"] = np.ascontiguousarray(w_mlp1.reshape(3, 3, HD, 1536), dtype=f)
    shared["b1"] = np.ascontiguousarray(
        b_mlp1.reshape(3, 12, HD).transpose(2, 0, 1), dtype=f)
    shared["w2"] = np.ascontiguousarray(w_mlp2.reshape(3, 12, HD, C), dtype=f)
    shared["b2r"] = np.ascontiguousarray(b_mlp2, dtype=f)
    shared["wpr"] = np.ascontiguousarray(w_proj.reshape(3, HD, C), dtype=f)
    shared["bpr"] = np.ascontiguousarray(b_proj.reshape(3, HD).T, dtype=f)

    in_maps = []
    for c in range(N_CORES):
        b, half = c // 2, c % 2
        r0 = half * R
        slab = np.zeros((C, RP, SP), f)
        glo, ghi = max(0, r0 - 8), min(HH, r0 + R + 8)
        plo = glo - (r0 - 8)
        slab[:, plo:plo + (ghi - glo), 8:104] = \
            np.asarray(x[b, glo:ghi], dtype=f).transpose(2, 0, 1)
        m = dict(shared)
        m["xp"] = np.ascontiguousarray(slab)
        in_maps.append(m)
    return in_maps


def kernel(**inputs):
    global _CACHED_NC
    if _CACHED_NC is None:
        _CACHED_NC = build_kernel()
    nc = _CACHED_NC
    in_maps = _prep_inputs(**{k: np.asarray(v) for k, v in inputs.items()})
    res = run_bass_kernel_spmd(nc, in_maps, core_ids=list(range(N_CORES)))
    out = np.empty((B, HH, WW, C), np.float32)
    for c in range(N_CORES):
        b, half = c // 2, c % 2
        slab = res.results[c]["out_cm"].reshape(C, R, 96)
        out[b, :, half * R:(half + 1) * R, :] = slab.transpose(2, 1, 0)
    return out


# revision 9
# speedup vs baseline: 1.0179x; 1.0179x over previous
"""Trainium2 Bass kernel for nn_MAMoE (conv-MoE -> row attention -> MLP-MoE).

Sharding: 8 cores = (batch b in 0..3) x (H-half in 0..1). All routing is
per-token; the reference's swapaxes(1,2) means attention row r produces
output column w=r, so each core independently computes the full pipeline
for its 48 attention rows and the host reassembles along W.

All large matmuls run as float32r (full-rate fp32 storage, ~1e-3 matmul
accuracy); small-N attention matmuls run fp32.
"""
import numpy as np

import concourse.bass as bass
import concourse.mybir as mybir
import concourse.tile as tile
from concourse import bacc
from concourse.bass_utils import run_bass_kernel_spmd
from concourse.masks import make_identity

F32 = mybir.dt.float32
F32R = mybir.dt.float32r

B, HH, WW, C = 4, 96, 96, 384
HD = 128
SCALE = float((HD // 3) ** -0.5)  # 42**-0.5
N_CORES = 8
R = 48            # attention rows per core
RP = 64           # padded rows in x slab (R + 2*8)
SP = 128          # padded W (96 + 2*8, padded to 512B row stride)
T = R * 96        # tokens per core = 4608
NT = 512          # tokens per MLP tile
NTILES = T // NT  # 9
GROUPS = R // 4   # 12 groups of 4 rows
GN = 4 * 96       # tokens per group = 384

TAPS_A = [
    [(dr, ds) for dr in (-1, 0, 1) for ds in (-1, 0, 1)],
    [(dr, 0) for dr in range(-4, 5)],
    [(0, ds) for ds in range(-4, 5)],
]
TAPS_B = [
    [(dr, ds) for dr in (-2, 0, 2) for ds in (-2, 0, 2)],
    [(dr, 0) for dr in range(-8, 9, 2)],
    [(0, ds) for ds in range(-8, 9, 2)],
]

_CACHED_NC = None


def build_kernel():
    nc = bacc.Bacc("TRN2", target_bir_lowering=False, debug=False)

    xp = nc.dram_tensor("xp", [C, RP, SP], F32R, kind="ExternalInput").ap()
    wca = nc.dram_tensor("wca", [3, 9, HD, HD], F32R, kind="ExternalInput").ap()
    wcb = nc.dram_tensor("wcb", [3, 9, HD, HD], F32R, kind="ExternalInput").ap()
    bca = nc.dram_tensor("bca", [HD, 3], F32, kind="ExternalInput").ap()
    bcb = nc.dram_tensor("bcb", [HD, 3], F32, kind="ExternalInput").ap()
    wgd = nc.dram_tensor("wgd", [3, HD, 1], F32R, kind="ExternalInput").ap()
    eb3 = nc.dram_tensor("eb3", [3, 384], F32R, kind="ExternalInput").ap()
    onesd = nc.dram_tensor("onesd", [1, HD], F32R, kind="ExternalInput").ap()
    wqk = nc.dram_tensor("wqk", [3, HD, 256], F32R, kind="ExternalInput").ap()
    wv = nc.dram_tensor("wv", [3, HD, HD], F32R, kind="ExternalInput").ap()
    wap = nc.dram_tensor("wap", [3, HD, HD], F32R, kind="ExternalInput").ap()
    bap = nc.dram_tensor("bap", [HD, 3], F32, kind="ExternalInput").ap()
    wgf = nc.dram_tensor("wgf", [3, HD, 3], F32R, kind="ExternalInput").ap()
    w1 = nc.dram_tensor("w1", [3, 3, HD, 1536], F32R, kind="ExternalInput").ap()
    b1 = nc.dram_tensor("b1", [HD, 3, 12], F32, kind="ExternalInput").ap()
    w2 = nc.dram_tensor("w2", [3, 12, HD, C], F32R, kind="ExternalInput").ap()
    b2r = nc.dram_tensor("b2r", [3, C], F32R, kind="ExternalInput").ap()
    wpr = nc.dram_tensor("wpr", [3, HD, C], F32R, kind="ExternalInput").ap()
    bpr = nc.dram_tensor("bpr", [HD, 3], F32, kind="ExternalInput").ap()
    out_cm = nc.dram_tensor("out_cm", [C, T], F32, kind="ExternalOutput").ap()

    with tile.TileContext(nc) as tc:
        with tc.tile_pool(name="consts", bufs=1) as consts, \
             tc.tile_pool(name="persist", bufs=1) as persist:
            ones_r = consts.tile([1, HD], F32R)
            nc.sync.dma_start(out=ones_r, in_=onesd)
            ident = consts.tile([HD, HD], F32)
            make_identity(nc, ident)

            bca_sb = persist.tile([HD, 3], F32)
            nc.sync.dma_start(out=bca_sb, in_=bca)
            bcb_sb = persist.tile([HD, 3], F32)
            nc.sync.dma_start(out=bcb_sb, in_=bcb)
            bap_sb = persist.tile([HD, 3], F32)
            nc.sync.dma_start(out=bap_sb, in_=bap)

            xc_t = [persist.tile([HD, T], F32R, tag=f"xc{i}", name=f"xc{i}") for i in range(3)]

            # ---------------- Phase A: conv MoE + attention per branch ----
            with tc.tile_pool(name="xpool", bufs=2) as xpool, \
                 tc.tile_pool(name="wpoolA", bufs=2) as wpoolA, \
                 tc.tile_pool(name="gpool", bufs=2) as gpool, \
                 tc.tile_pool(name="psA", bufs=8, space="PSUM") as psA:
                for i in range(3):
                    xp_sb = xpool.tile([HD, RP, SP], F32R, tag="xp")
                    nc.sync.dma_start(out=xp_sb, in_=xp[i * HD:(i + 1) * HD])
                    wca_sb = wpoolA.tile([HD, 9, HD], F32R, tag="wca")
                    nc.sync.dma_start(out=wca_sb, in_=wca[i].rearrange("a p b -> p a b"))
                    wcb_sb = wpoolA.tile([HD, 9, HD], F32R, tag="wcb")
                    nc.sync.dma_start(out=wcb_sb, in_=wcb[i].rearrange("a p b -> p a b"))
                    wgd_sb = wpoolA.tile([HD, 1], F32R, tag="wgd")
                    nc.sync.dma_start(out=wgd_sb, in_=wgd[i])
                    wqk_sb = wpoolA.tile([HD, 256], F32R, tag="wqk")
                    nc.sync.dma_start(out=wqk_sb, in_=wqk[i])
                    wv_sb = wpoolA.tile([HD, HD], F32R, tag="wv")
                    nc.sync.dma_start(out=wv_sb, in_=wv[i])
                    wap_sb = wpoolA.tile([HD, HD], F32R, tag="wap")
                    nc.sync.dma_start(out=wap_sb, in_=wap[i])

                    for g in range(GROUPS):
                        rb = 8 + 4 * g  # xp row of first moe row in group
                        # two expert convs, 9 taps each, accumulated in PSUM
                        pa = psA.tile([HD, GN], F32, tag="ps")
                        for ti, (dr, ds) in enumerate(TAPS_A[i]):
                            nc.tensor.matmul(
                                pa, wca_sb[:, ti, :],
                                xp_sb[:, rb + dr: rb + dr + 4, 8 + ds: 104 + ds],
                                start=(ti == 0), stop=(ti == 8))
                        pb = psA.tile([HD, GN], F32, tag="ps")
                        for ti, (dr, ds) in enumerate(TAPS_B[i]):
                            nc.tensor.matmul(
                                pb, wcb_sb[:, ti, :],
                                xp_sb[:, rb + dr: rb + dr + 4, 8 + ds: 104 + ds],
                                start=(ti == 0), stop=(ti == 8))
                        # 2-expert gate: g0 = sigmoid(-(l1 - l0))
                        plg = psA.tile([1, GN], F32, tag="ps")
                        nc.tensor.matmul(plg, wgd_sb,
                                         xp_sb[:, rb: rb + 4, 8:104],
                                         start=True, stop=True)
                        ex = gpool.tile([1, GN], F32, tag="ex")
                        nc.scalar.activation(ex, plg,
                                             mybir.ActivationFunctionType.Exp,
                                             scale=1.0)
                        exp1 = gpool.tile([1, GN], F32, tag="exp1")
                        nc.vector.tensor_scalar_add(exp1, ex, 1.0)
                        g0 = gpool.tile([1, GN], F32R, tag="g0")
                        with nc.allow_low_precision(reason="f32r gate"):
                            nc.vector.reciprocal(g0, exp1)
                        pgb = psA.tile([HD, GN], F32, tag="ps")
                        nc.tensor.matmul(pgb, ones_r, g0, start=True, stop=True)
                        # moe = g0*(ca - cb) + cb  (bias-add fused on ACT)
                        ca = gpool.tile([HD, GN], F32, tag="ca")
                        nc.scalar.activation(ca, pa,
                                             mybir.ActivationFunctionType.Identity,
                                             bias=bca_sb[:, i:i + 1])
                        cb = gpool.tile([HD, GN], F32, tag="cb")
                        nc.scalar.activation(cb, pb,
                                             mybir.ActivationFunctionType.Identity,
                                             bias=bcb_sb[:, i:i + 1])
                        dd = gpool.tile([HD, GN], F32, tag="dd")
                        nc.vector.tensor_sub(dd, ca, cb)
                        d2 = gpool.tile([HD, GN], F32, tag="d2")
                        nc.vector.tensor_mul(d2, dd, pgb)
                        moe = gpool.tile([HD, GN], F32R, tag="moe")
                        nc.vector.tensor_add(moe, d2, cb)
                        # q, k projections (channel-major)
                        pq = psA.tile([HD, GN], F32, tag="ps")
                        nc.tensor.matmul(pq, wqk_sb[:, 0:HD], moe,
                                         start=True, stop=True)
                        q_sb = gpool.tile([HD, GN], F32, tag="q")
                        nc.vector.tensor_copy(q_sb, pq)
                        pk = psA.tile([HD, GN], F32, tag="ps")
                        nc.tensor.matmul(pk, wqk_sb[:, HD:256], moe,
                                         start=True, stop=True)
                        k_sb = gpool.tile([HD, GN], F32, tag="k")
                        nc.vector.tensor_copy(k_sb, pk)
                        # vT per row: [96 tok, 128 ch]
                        pvt = psA.tile([96, 4 * HD], F32, tag="ps")
                        for j in range(4):
                            nc.tensor.matmul(pvt[:, j * HD:(j + 1) * HD],
                                             moe[:, j * 96:(j + 1) * 96], wv_sb,
                                             start=True, stop=True)
                        vt_sb = gpool.tile([96, 4 * HD], F32, tag="vt")
                        nc.vector.tensor_copy(vt_sb, pvt)
                        # scores + softmax (no max-sub: logits are tiny)
                        psc = psA.tile([96, GN], F32, tag="ps")
                        for j in range(4):
                            nc.tensor.matmul(psc[:, j * 96:(j + 1) * 96],
                                             q_sb[:, j * 96:(j + 1) * 96],
                                             k_sb[:, j * 96:(j + 1) * 96],
                                             start=True, stop=True)
                        probs = gpool.tile([96, GN], F32, tag="probs")
                        nc.scalar.activation(probs, psc,
                                             mybir.ActivationFunctionType.Exp,
                                             scale=SCALE)
                        zsum = gpool.tile([96, 4], F32, tag="zsum")
                        nc.vector.tensor_reduce(
                            zsum, probs.rearrange("p (j q) -> p j q", q=96),
                            axis=mybir.AxisListType.X, op=mybir.AluOpType.add)
                        rec = gpool.tile([96, 4], F32, tag="rec")
                        nc.vector.reciprocal(rec, zsum)
                        pn = gpool.tile([96, GN], F32, tag="pn")
                        for j in range(4):
                            nc.vector.tensor_scalar_mul(
                                pn[:, j * 96:(j + 1) * 96],
                                probs[:, j * 96:(j + 1) * 96],
                                rec[:, j:j + 1])
                        ppt = psA.tile([96, GN], F32, tag="ps")
                        for j in range(4):
                            nc.tensor.transpose(ppt[:, j * 96:(j + 1) * 96],
                                                pn[:, j * 96:(j + 1) * 96],
                                                ident[:96, :96])
                        pt_sb = gpool.tile([96, GN], F32, tag="pt")
                        nc.vector.tensor_copy(pt_sb, ppt)
                        po = psA.tile([HD, GN], F32, tag="ps")
                        for j in range(4):
                            nc.tensor.matmul(po[:, j * 96:(j + 1) * 96],
                                             vt_sb[:, j * HD:(j + 1) * HD],
                                             pt_sb[:, j * 96:(j + 1) * 96],
                                             start=True, stop=True)
                        og = gpool.tile([HD, GN], F32R, tag="og")
                        nc.vector.tensor_copy(og, po)
                        pap2 = psA.tile([HD, GN], F32, tag="ps")
                        nc.tensor.matmul(pap2, wap_sb, og, start=True, stop=True)
                        nc.scalar.activation(
                            xc_t[i][:, g * GN:(g + 1) * GN], pap2,
                            mybir.ActivationFunctionType.Identity,
                            bias=bap_sb[:, i:i + 1])

            # ---------------- Phase B: final MLP MoE + proj ---------------
            with tc.tile_pool(name="wpoolB", bufs=1) as wpoolB, \
                 tc.tile_pool(name="bpool", bufs=2) as bpool, \
                 tc.tile_pool(name="spool", bufs=1) as spool, \
                 tc.tile_pool(name="psL", bufs=3, space="PSUM") as psL, \
                 tc.tile_pool(name="psB", bufs=5, space="PSUM") as psB:
                b1_sb = wpoolB.tile([HD, 3, 12], F32)
                nc.sync.dma_start(out=b1_sb, in_=b1)
                b2r_sb = wpoolB.tile([3, C], F32R)
                nc.sync.dma_start(out=b2r_sb, in_=b2r)
                wgf_sb = wpoolB.tile([HD, 3, 3], F32R)
                nc.sync.dma_start(out=wgf_sb, in_=wgf.rearrange("a p b -> p a b"))
                wpr_sb = wpoolB.tile([HD, 3, C], F32R)
                nc.sync.dma_start(out=wpr_sb, in_=wpr.rearrange("a p b -> p a b"))
                bpr_sb = wpoolB.tile([HD, 3], F32)
                nc.sync.dma_start(out=bpr_sb, in_=bpr)
                eb3_sb = wpoolB.tile([3, 384], F32R)
                nc.sync.dma_start(out=eb3_sb, in_=eb3)
                w1_sb = []
                w2_sb = []
                for e in range(3):
                    t1 = wpoolB.tile([HD, 3, 1536], F32R, tag=f"w1_{e}", name=f"w1_{e}")
                    nc.sync.dma_start(out=t1, in_=w1[e].rearrange("a p b -> p a b"))
                    w1_sb.append(t1)
                    t2 = wpoolB.tile([HD, 12, C], F32R, tag=f"w2_{e}", name=f"w2_{e}")
                    nc.sync.dma_start(out=t2, in_=w2[e].rearrange("a p b -> p a b"))
                    w2_sb.append(t2)

                for t in range(NTILES):
                    t0 = t * NT
                    # gating logits -> [3, NT] psum
                    plg = psB.tile([3, NT], F32, tag="ps")
                    for kc in range(3):
                        nc.tensor.matmul(plg, wgf_sb[:, kc, :],
                                         xc_t[kc][:, t0:t0 + NT],
                                         start=(kc == 0), stop=(kc == 2))
                    lsb = spool.tile([3, NT], F32, tag="lsb")
                    nc.vector.tensor_copy(lsb, plg)
                    # token-major logits [128, 4, 3]
                    plt = psB.tile([HD, 12], F32, tag="ps")
                    for t4 in range(4):
                        nc.tensor.transpose(plt[:, t4 * 3:(t4 + 1) * 3],
                                            lsb[:, t4 * HD:(t4 + 1) * HD],
                                            ident[:3, :3])
                    lt = spool.tile([HD, 12], F32, tag="lt")
                    nc.vector.tensor_copy(lt, plt)
                    e_sb = spool.tile([HD, 12], F32, tag="e_sb")
                    nc.scalar.activation(e_sb, lt,
                                         mybir.ActivationFunctionType.Exp)
                    e3 = e_sb.rearrange("p (j e) -> p j e", e=3)
                    s4 = spool.tile([HD, 4], F32, tag="s4")
                    nc.vector.tensor_reduce(s4, e3, axis=mybir.AxisListType.X,
                                            op=mybir.AluOpType.add)
                    mn = spool.tile([HD, 4], F32, tag="mn")
                    nc.vector.tensor_reduce(mn, e3, axis=mybir.AxisListType.X,
                                            op=mybir.AluOpType.min)
                    den = spool.tile([HD, 4], F32, tag="den")
                    nc.vector.tensor_sub(den, s4, mn)
                    rec = spool.tile([HD, 4], F32, tag="recb")
                    nc.vector.reciprocal(rec, den)
                    gfin = spool.tile([HD, 12], F32, tag="gfin")
                    nmask = spool.tile([HD, 12], F32, tag="nmask")
                    for t4 in range(4):
                        sl = slice(t4 * 3, (t4 + 1) * 3)
                        nc.vector.tensor_scalar_mul(gfin[:, sl], e_sb[:, sl],
                                                    rec[:, t4:t4 + 1])
                        nc.vector.tensor_scalar(nmask[:, sl], e_sb[:, sl],
                                                mn[:, t4:t4 + 1], None,
                                                op0=mybir.AluOpType.not_equal)
                    gm = spool.tile([HD, 12], F32, tag="gm")
                    nc.vector.tensor_mul(gm, gfin, nmask)
                    # back to expert-major [3, NT]
                    pgt = psB.tile([3, NT], F32, tag="ps")
                    for t4 in range(4):
                        nc.tensor.transpose(pgt[:, t4 * HD:(t4 + 1) * HD],
                                            gm[:, t4 * 3:(t4 + 1) * 3],
                                            ident)
                    gates_r = spool.tile([3, NT], F32R, tag="gates")
                    nc.vector.tensor_copy(gates_r, pgt)

                    pd = [psL.tile([HD, NT], F32, tag="down", name=f"pd{_i}") for _i in range(3)]
                    for e in range(3):
                        pgb = psB.tile([HD, NT], F32, tag="ps")
                        nc.tensor.matmul(pgb, eb3_sb[:, e * HD:(e + 1) * HD],
                                         gates_r, start=True, stop=True)
                        for m in range(12):
                            pu = psB.tile([HD, NT], F32, tag="ps")
                            for kc in range(3):
                                nc.tensor.matmul(
                                    pu, w1_sb[e][:, kc, m * HD:(m + 1) * HD],
                                    xc_t[kc][:, t0:t0 + NT],
                                    start=(kc == 0), stop=(kc == 2))
                            h = bpool.tile([HD, NT], F32, tag="h")
                            nc.scalar.activation(
                                h, pu, mybir.ActivationFunctionType.Gelu,
                                bias=b1_sb[:, e, m:m + 1])
                            hs = bpool.tile([HD, NT], F32R, tag="hs")
                            nc.vector.tensor_mul(hs, h, pgb)
                            for mp in range(3):
                                nc.tensor.matmul(
                                    pd[mp], w2_sb[e][:, m, mp * HD:(mp + 1) * HD],
                                    hs, start=(e == 0 and m == 0), stop=False)
                    for mp in range(3):
                        nc.tensor.matmul(pd[mp], b2r_sb[:, mp * HD:(mp + 1) * HD],
                                         gates_r, start=False, stop=True)
                    dsb = []
                    for mp in range(3):
                        dt_ = spool.tile([HD, NT], F32R, tag=f"dsb{mp}", name=f"dsb{mp}")
                        nc.vector.tensor_copy(dt_, pd[mp])
                        dsb.append(dt_)
                    for mp in range(3):
                        ppj = psB.tile([HD, NT], F32, tag="ps")
                        for kc in range(3):
                            nc.tensor.matmul(ppj,
                                             wpr_sb[:, kc, mp * HD:(mp + 1) * HD],
                                             dsb[kc],
                                             start=(kc == 0), stop=(kc == 2))
                        osb = bpool.tile([HD, NT], F32, tag="osb")
                        nc.scalar.activation(osb, ppj,
                                             mybir.ActivationFunctionType.Identity,
                                             bias=bpr_sb[:, mp:mp + 1])
                        nc.sync.dma_start(
                            out=out_cm[mp * HD:(mp + 1) * HD, t0:t0 + NT],
                            in_=osb)
    nc.compile()
    return nc


def _prep_inputs(x, w_e1, b_e1, w_e2, b_e2, w_e3, b_e3, w_e4, b_e4, w_e5, b_e5,
                 w_e6, b_e6, wg1, wg2, wg3, w_qkv, w_attn_proj, b_attn_proj,
                 wg_final, w_mlp1, b_mlp1, w_mlp2, b_mlp2, w_proj, b_proj):
    f = np.float32
    shared = {}
    shared["wca"] = np.ascontiguousarray(np.stack([
        w_e1.reshape(9, HD, HD), w_e3.reshape(9, HD, HD),
        w_e5.reshape(9, HD, HD)]), dtype=f)
    shared["wcb"] = np.ascontiguousarray(np.stack([
        w_e2.reshape(9, HD, HD), w_e4.reshape(9, HD, HD),
        w_e6.reshape(9, HD, HD)]), dtype=f)
    shared["bca"] = np.ascontiguousarray(np.stack([b_e1, b_e3, b_e5], axis=1), dtype=f)
    shared["bcb"] = np.ascontiguousarray(np.stack([b_e2, b_e4, b_e6], axis=1), dtype=f)
    wgs = np.stack([wg1, wg2, wg3])
    shared["wgd"] = np.ascontiguousarray(
        (wgs[:, :, 1] - wgs[:, :, 0])[:, :, None], dtype=f)
    eb3 = np.zeros((3, 384), f)
    for e in range(3):
        eb3[e, e * 128:(e + 1) * 128] = 1.0
    shared["eb3"] = eb3
    shared["onesd"] = np.ones((1, 128), f)
    shared["wqk"] = np.ascontiguousarray(w_qkv[:, :, :256], dtype=f)
    shared["wv"] = np.ascontiguousarray(w_qkv[:, :, 256:], dtype=f)
    shared["wap"] = np.ascontiguousarray(w_attn_proj, dtype=f)
    shared["bap"] = np.ascontiguousarray(b_attn_proj.T, dtype=f)
    shared["wgf"] = np.ascontiguousarray(wg_final.reshape(3, HD, 3), dtype=f)
    shared["w1"] = np.ascontiguousarray(w_mlp1.reshape(3, 3, HD, 1536), dtype=f)
    shared["b1"] = np.ascontiguousarray(
        b_mlp1.reshape(3, 12, HD).transpose(2, 0, 1), dtype=f)
    shared["w2"] = np.ascontiguousarray(w_mlp2.reshape(3, 12, HD, C), dtype=f)
    shared["b2r"] = np.ascontiguousarray(b_mlp2, dtype=f)
    shared["wpr"] = np.ascontiguousarray(w_proj.reshape(3, HD, C), dtype=f)
    shared["bpr"] = np.ascontiguousarray(b_proj.reshape(3, HD).T, dtype=f)

    in_maps = []
    for c in range(N_CORES):
        b, half = c // 2, c % 2
        r0 = half * R
        slab = np.zeros((C, RP, SP), f)
        glo, ghi = max(0, r0 - 8), min(HH, r0 + R + 8)
        plo = glo - (r0 - 8)
        slab[:, plo:plo + (ghi - glo), 8:104] = \
            np.asarray(x[b, glo:ghi], dtype=f).transpose(2, 0, 1)
        m = dict(shared)
        m["xp"] = np.ascontiguousarray(slab)
        in_maps.append(m)
    return in_maps


def kernel(**inputs):
    global _CACHED_NC
    if _CACHED_NC is None:
        _CACHED_NC = build_kernel()
    nc = _CACHED_NC
    in_maps = _prep_inputs(**{k: np.asarray(v) for k, v in inputs.items()})
    res = run_bass_kernel_spmd(nc, in_maps, core_ids=list(range(N_CORES)))
    out = np.empty((B, HH, WW, C), np.float32)
    for c in range(N_CORES):
        b, half = c // 2, c % 2
        slab = res.results[c]["out_cm"].reshape(C, R, 96)
        out[b, :, half * R:(half + 1) * R, :] = slab.transpose(2, 1, 0)
    return out


# revision 10
# speedup vs baseline: 1.0311x; 1.0130x over previous
"""Trainium2 Bass kernel for nn_MAMoE (conv-MoE -> row attention -> MLP-MoE).

Sharding: 8 cores = (batch b in 0..3) x (H-half in 0..1). All routing is
per-token; the reference's swapaxes(1,2) means attention row r produces
output column w=r, so each core independently computes the full pipeline
for its 48 attention rows and the host reassembles along W.

All large matmuls run as float32r (full-rate fp32 storage, ~1e-3 matmul
accuracy); small-N attention matmuls run fp32.
"""
import numpy as np

import concourse.bass as bass
import concourse.mybir as mybir
import concourse.tile as tile
from concourse import bacc
from concourse.bass_utils import run_bass_kernel_spmd
from concourse.masks import make_identity

F32 = mybir.dt.float32
F32R = mybir.dt.float32r

B, HH, WW, C = 4, 96, 96, 384
HD = 128
SCALE = float((HD // 3) ** -0.5)  # 42**-0.5
N_CORES = 8
R = 48            # attention rows per core
RP = 65           # padded rows (R + 2*8) + 1 slack row for shifted flat reads
SP = 128          # padded W (96 + 2*8, padded to 512B row stride)
T = R * 96        # tokens per core = 4608
NT = 512          # tokens per MLP tile
NTILES = T // NT  # 9
GROUPS = R // 4   # 12 groups of 4 rows
GN = 4 * 96       # tokens per group = 384

TAPS_A = [
    [(dr, ds) for dr in (-1, 0, 1) for ds in (-1, 0, 1)],
    [(dr, 0) for dr in range(-4, 5)],
    [(0, ds) for ds in range(-4, 5)],
]
TAPS_B = [
    [(dr, ds) for dr in (-2, 0, 2) for ds in (-2, 0, 2)],
    [(dr, 0) for dr in range(-8, 9, 2)],
    [(0, ds) for ds in range(-8, 9, 2)],
]

_CACHED_NC = None


def build_kernel():
    nc = bacc.Bacc("TRN2", target_bir_lowering=False, debug=False)

    xp = nc.dram_tensor("xp", [C, RP, SP], F32R, kind="ExternalInput").ap()
    wca = nc.dram_tensor("wca", [3, 9, HD, HD], F32R, kind="ExternalInput").ap()
    wcb = nc.dram_tensor("wcb", [3, 9, HD, HD], F32R, kind="ExternalInput").ap()
    bca = nc.dram_tensor("bca", [HD, 3], F32, kind="ExternalInput").ap()
    bcb = nc.dram_tensor("bcb", [HD, 3], F32, kind="ExternalInput").ap()
    wgd = nc.dram_tensor("wgd", [3, HD, 1], F32R, kind="ExternalInput").ap()
    eb3 = nc.dram_tensor("eb3", [3, 384], F32R, kind="ExternalInput").ap()
    onesd = nc.dram_tensor("onesd", [1, HD], F32R, kind="ExternalInput").ap()
    wqk = nc.dram_tensor("wqk", [3, HD, 256], F32R, kind="ExternalInput").ap()
    wv = nc.dram_tensor("wv", [3, HD, HD], F32R, kind="ExternalInput").ap()
    wap = nc.dram_tensor("wap", [3, HD, HD], F32R, kind="ExternalInput").ap()
    bap = nc.dram_tensor("bap", [HD, 3], F32, kind="ExternalInput").ap()
    wgf = nc.dram_tensor("wgf", [3, HD, 3], F32R, kind="ExternalInput").ap()
    w1 = nc.dram_tensor("w1", [3, 3, HD, 1536], F32R, kind="ExternalInput").ap()
    b1 = nc.dram_tensor("b1", [HD, 3, 12], F32, kind="ExternalInput").ap()
    w2 = nc.dram_tensor("w2", [3, 12, HD, C], F32R, kind="ExternalInput").ap()
    b2r = nc.dram_tensor("b2r", [3, C], F32R, kind="ExternalInput").ap()
    wpr = nc.dram_tensor("wpr", [3, HD, C], F32R, kind="ExternalInput").ap()
    bpr = nc.dram_tensor("bpr", [HD, 3], F32, kind="ExternalInput").ap()
    out_cm = nc.dram_tensor("out_cm", [C, T], F32, kind="ExternalOutput").ap()

    with tile.TileContext(nc) as tc:
        with tc.tile_pool(name="consts", bufs=1) as consts, \
             tc.tile_pool(name="persist", bufs=1) as persist:
            ones_r = consts.tile([1, HD], F32R)
            nc.sync.dma_start(out=ones_r, in_=onesd)
            ident = consts.tile([HD, HD], F32)
            make_identity(nc, ident)

            bca_sb = persist.tile([HD, 3], F32)
            nc.sync.dma_start(out=bca_sb, in_=bca)
            bcb_sb = persist.tile([HD, 3], F32)
            nc.sync.dma_start(out=bcb_sb, in_=bcb)
            bap_sb = persist.tile([HD, 3], F32)
            nc.sync.dma_start(out=bap_sb, in_=bap)

            xc_t = [persist.tile([HD, T], F32R, tag=f"xc{i}", name=f"xc{i}") for i in range(3)]

            # ---------------- Phase A: conv MoE + attention per branch ----
            with tc.tile_pool(name="xpool", bufs=2) as xpool, \
                 tc.tile_pool(name="wpoolA", bufs=2) as wpoolA, \
                 tc.tile_pool(name="gpool", bufs=2) as gpool, \
                 tc.tile_pool(name="psA", bufs=8, space="PSUM") as psA:
                for i in range(3):
                    xp_sb = xpool.tile([HD, RP, SP], F32R, tag="xp")
                    nc.sync.dma_start(out=xp_sb, in_=xp[i * HD:(i + 1) * HD])
                    wca_sb = wpoolA.tile([HD, 9, HD], F32R, tag="wca")
                    nc.sync.dma_start(out=wca_sb, in_=wca[i].rearrange("a p b -> p a b"))
                    wcb_sb = wpoolA.tile([HD, 9, HD], F32R, tag="wcb")
                    nc.sync.dma_start(out=wcb_sb, in_=wcb[i].rearrange("a p b -> p a b"))
                    wgd_sb = wpoolA.tile([HD, 1], F32R, tag="wgd")
                    nc.sync.dma_start(out=wgd_sb, in_=wgd[i])
                    wqk_sb = wpoolA.tile([HD, 256], F32R, tag="wqk")
                    nc.sync.dma_start(out=wqk_sb, in_=wqk[i])
                    wv_sb = wpoolA.tile([HD, HD], F32R, tag="wv")
                    nc.sync.dma_start(out=wv_sb, in_=wv[i])
                    wap_sb = wpoolA.tile([HD, HD], F32R, tag="wap")
                    nc.sync.dma_start(out=wap_sb, in_=wap[i])

                    xpf = xp_sb.rearrange("p r s -> p (r s)")
                    for g in range(GROUPS):
                        rb = 8 + 4 * g  # xp row of first moe row in group
                        fo = rb * SP    # flat offset of group start
                        # gate logits first so ACT/DVE gate math hides
                        # under the conv matmuls
                        plg = psA.tile([1, NT], F32, tag="ps")
                        nc.tensor.matmul(plg, wgd_sb, xpf[:, fo:fo + NT],
                                         start=True, stop=True)
                        ex = gpool.tile([1, NT], F32, tag="ex")
                        nc.scalar.activation(ex, plg,
                                             mybir.ActivationFunctionType.Exp,
                                             scale=1.0)
                        exp1 = gpool.tile([1, NT], F32, tag="exp1")
                        nc.vector.tensor_scalar_add(exp1, ex, 1.0)
                        g0 = gpool.tile([1, NT], F32R, tag="g0")
                        with nc.allow_low_precision(reason="f32r gate"):
                            nc.vector.reciprocal(g0, exp1)
                        # two expert convs over full padded rows (N=512,
                        # contiguous; shifted tails land in pad columns)
                        pa = psA.tile([HD, NT], F32, tag="ps")
                        for ti, (dr, ds) in enumerate(TAPS_A[i]):
                            nc.tensor.matmul(
                                pa, wca_sb[:, ti, :],
                                xpf[:, fo + dr * SP + ds: fo + dr * SP + ds + NT],
                                start=(ti == 0), stop=(ti == 8))
                        pb = psA.tile([HD, NT], F32, tag="ps")
                        for ti, (dr, ds) in enumerate(TAPS_B[i]):
                            nc.tensor.matmul(
                                pb, wcb_sb[:, ti, :],
                                xpf[:, fo + dr * SP + ds: fo + dr * SP + ds + NT],
                                start=(ti == 0), stop=(ti == 8))
                        pgb = psA.tile([HD, NT], F32, tag="ps")
                        nc.tensor.matmul(pgb, ones_r, g0, start=True, stop=True)
                        # moe = g0*(ca - cb) + cb  (bias-add fused on ACT)
                        ca = gpool.tile([HD, NT], F32, tag="ca")
                        nc.scalar.activation(ca, pa,
                                             mybir.ActivationFunctionType.Identity,
                                             bias=bca_sb[:, i:i + 1])
                        cb = gpool.tile([HD, NT], F32, tag="cb")
                        nc.scalar.activation(cb, pb,
                                             mybir.ActivationFunctionType.Identity,
                                             bias=bcb_sb[:, i:i + 1])
                        dd = gpool.tile([HD, NT], F32, tag="dd")
                        nc.vector.tensor_sub(dd, ca, cb)
                        d2 = gpool.tile([HD, NT], F32, tag="d2")
                        nc.vector.tensor_mul(d2, dd, pgb)
                        moe = gpool.tile([HD, NT], F32R, tag="moe")
                        nc.vector.tensor_add(moe, d2, cb)
                        # q, k projections (channel-major, full rows)
                        pq = psA.tile([HD, NT], F32, tag="ps")
                        nc.tensor.matmul(pq, wqk_sb[:, 0:HD], moe,
                                         start=True, stop=True)
                        q_sb = gpool.tile([HD, NT], F32, tag="q")
                        nc.vector.tensor_copy(q_sb, pq)
                        pk = psA.tile([HD, NT], F32, tag="ps")
                        nc.tensor.matmul(pk, wqk_sb[:, HD:256], moe,
                                         start=True, stop=True)
                        k_sb = gpool.tile([HD, NT], F32, tag="k")
                        nc.vector.tensor_copy(k_sb, pk)
                        # vT per row: [96 tok, 128 ch]
                        pvt = psA.tile([96, 4 * HD], F32, tag="ps")
                        for j in range(4):
                            nc.tensor.matmul(pvt[:, j * HD:(j + 1) * HD],
                                             moe[:, j * SP + 8: j * SP + 104],
                                             wv_sb, start=True, stop=True)
                        vt_sb = gpool.tile([96, 4 * HD], F32, tag="vt")
                        nc.vector.tensor_copy(vt_sb, pvt)
                        # scores + softmax (no max-sub: logits are tiny)
                        psc = psA.tile([96, GN], F32, tag="ps")
                        for j in range(4):
                            nc.tensor.matmul(psc[:, j * 96:(j + 1) * 96],
                                             q_sb[:, j * SP + 8: j * SP + 104],
                                             k_sb[:, j * SP + 8: j * SP + 104],
                                             start=True, stop=True)
                        probs = gpool.tile([96, GN], F32, tag="probs")
                        nc.scalar.activation(probs, psc,
                                             mybir.ActivationFunctionType.Exp,
                                             scale=SCALE)
                        zsum = gpool.tile([96, 4], F32, tag="zsum")
                        nc.vector.tensor_reduce(
                            zsum, probs.rearrange("p (j q) -> p j q", q=96),
                            axis=mybir.AxisListType.X, op=mybir.AluOpType.add)
                        rec = gpool.tile([96, 4], F32, tag="rec")
                        nc.vector.reciprocal(rec, zsum)
                        pn = gpool.tile([96, GN], F32, tag="pn")
                        for j in range(4):
                            nc.vector.tensor_scalar_mul(
                                pn[:, j * 96:(j + 1) * 96],
                                probs[:, j * 96:(j + 1) * 96],
                                rec[:, j:j + 1])
                        ppt = psA.tile([96, GN], F32, tag="ps")
                        for j in range(4):
                            nc.tensor.transpose(ppt[:, j * 96:(j + 1) * 96],
                                                pn[:, j * 96:(j + 1) * 96],
                                                ident[:96, :96])
                        pt_sb = gpool.tile([96, GN], F32, tag="pt")
                        nc.vector.tensor_copy(pt_sb, ppt)
                        po = psA.tile([HD, GN], F32, tag="ps")
                        for j in range(4):
                            nc.tensor.matmul(po[:, j * 96:(j + 1) * 96],
                                             vt_sb[:, j * HD:(j + 1) * HD],
                                             pt_sb[:, j * 96:(j + 1) * 96],
                                             start=True, stop=True)
                        og = gpool.tile([HD, GN], F32R, tag="og")
                        nc.vector.tensor_copy(og, po)
                        pap2 = psA.tile([HD, GN], F32, tag="ps")
                        nc.tensor.matmul(pap2, wap_sb, og, start=True, stop=True)
                        nc.scalar.activation(
                            xc_t[i][:, g * GN:(g + 1) * GN], pap2,
                            mybir.ActivationFunctionType.Identity,
                            bias=bap_sb[:, i:i + 1])

            # ---------------- Phase B: final MLP MoE + proj ---------------
            with tc.tile_pool(name="wpoolB", bufs=1) as wpoolB, \
                 tc.tile_pool(name="bpool", bufs=2) as bpool, \
                 tc.tile_pool(name="spool", bufs=1) as spool, \
                 tc.tile_pool(name="psL", bufs=3, space="PSUM") as psL, \
                 tc.tile_pool(name="psB", bufs=5, space="PSUM") as psB:
                b1_sb = wpoolB.tile([HD, 3, 12], F32)
                nc.sync.dma_start(out=b1_sb, in_=b1)
                b2r_sb = wpoolB.tile([3, C], F32R)
                nc.sync.dma_start(out=b2r_sb, in_=b2r)
                wgf_sb = wpoolB.tile([HD, 3, 3], F32R)
                nc.sync.dma_start(out=wgf_sb, in_=wgf.rearrange("a p b -> p a b"))
                wpr_sb = wpoolB.tile([HD, 3, C], F32R)
                nc.sync.dma_start(out=wpr_sb, in_=wpr.rearrange("a p b -> p a b"))
                bpr_sb = wpoolB.tile([HD, 3], F32)
                nc.sync.dma_start(out=bpr_sb, in_=bpr)
                eb3_sb = wpoolB.tile([3, 384], F32R)
                nc.sync.dma_start(out=eb3_sb, in_=eb3)
                w1_sb = []
                w2_sb = []
                for e in range(3):
                    t1 = wpoolB.tile([HD, 3, 1536], F32R, tag=f"w1_{e}", name=f"w1_{e}")
                    nc.sync.dma_start(out=t1, in_=w1[e].rearrange("a p b -> p a b"))
                    w1_sb.append(t1)
                    t2 = wpoolB.tile([HD, 12, C], F32R, tag=f"w2_{e}", name=f"w2_{e}")
                    nc.sync.dma_start(out=t2, in_=w2[e].rearrange("a p b -> p a b"))
                    w2_sb.append(t2)

                for t in range(NTILES):
                    t0 = t * NT
                    # gating logits -> [3, NT] psum
                    plg = psB.tile([3, NT], F32, tag="ps")
                    for kc in range(3):
                        nc.tensor.matmul(plg, wgf_sb[:, kc, :],
                                         xc_t[kc][:, t0:t0 + NT],
                                         start=(kc == 0), stop=(kc == 2))
                    lsb = spool.tile([3, NT], F32, tag="lsb")
                    nc.vector.tensor_copy(lsb, plg)
                    # token-major logits [128, 4, 3]
                    plt = psB.tile([HD, 12], F32, tag="ps")
                    for t4 in range(4):
                        nc.tensor.transpose(plt[:, t4 * 3:(t4 + 1) * 3],
                                            lsb[:, t4 * HD:(t4 + 1) * HD],
                                            ident[:3, :3])
                    lt = spool.tile([HD, 12], F32, tag="lt")
                    nc.vector.tensor_copy(lt, plt)
                    e_sb = spool.tile([HD, 12], F32, tag="e_sb")
                    nc.scalar.activation(e_sb, lt,
                                         mybir.ActivationFunctionType.Exp)
                    e3 = e_sb.rearrange("p (j e) -> p j e", e=3)
                    s4 = spool.tile([HD, 4], F32, tag="s4")
                    nc.vector.tensor_reduce(s4, e3, axis=mybir.AxisListType.X,
                                            op=mybir.AluOpType.add)
                    mn = spool.tile([HD, 4], F32, tag="mn")
                    nc.vector.tensor_reduce(mn, e3, axis=mybir.AxisListType.X,
                                            op=mybir.AluOpType.min)
                    den = spool.tile([HD, 4], F32, tag="den")
                    nc.vector.tensor_sub(den, s4, mn)
                    rec = spool.tile([HD, 4], F32, tag="recb")
                    nc.vector.reciprocal(rec, den)
                    gfin = spool.tile([HD, 12], F32, tag="gfin")
                    nmask = spool.tile([HD, 12], F32, tag="nmask")
                    for t4 in range(4):
                        sl = slice(t4 * 3, (t4 + 1) * 3)
                        nc.vector.tensor_scalar_mul(gfin[:, sl], e_sb[:, sl],
                                                    rec[:, t4:t4 + 1])
                        nc.vector.tensor_scalar(nmask[:, sl], e_sb[:, sl],
                                                mn[:, t4:t4 + 1], None,
                                                op0=mybir.AluOpType.not_equal)
                    gm = spool.tile([HD, 12], F32, tag="gm")
                    nc.vector.tensor_mul(gm, gfin, nmask)
                    # back to expert-major [3, NT]
                    pgt = psB.tile([3, NT], F32, tag="ps")
                    for t4 in range(4):
                        nc.tensor.transpose(pgt[:, t4 * HD:(t4 + 1) * HD],
                                            gm[:, t4 * 3:(t4 + 1) * 3],
                                            ident)
                    gates_r = spool.tile([3, NT], F32R, tag="gates")
                    nc.vector.tensor_copy(gates_r, pgt)

                    pd = [psL.tile([HD, NT], F32, tag="down", name=f"pd{_i}") for _i in range(3)]
                    for e in range(3):
                        pgb = psB.tile([HD, NT], F32, tag="ps")
                        nc.tensor.matmul(pgb, eb3_sb[:, e * HD:(e + 1) * HD],
                                         gates_r, start=True, stop=True)
                        for m in range(12):
                            pu = psB.tile([HD, NT], F32, tag="ps")
                            for kc in range(3):
                                nc.tensor.matmul(
                                    pu, w1_sb[e][:, kc, m * HD:(m + 1) * HD],
                                    xc_t[kc][:, t0:t0 + NT],
                                    start=(kc == 0), stop=(kc == 2))
                            h = bpool.tile([HD, NT], F32, tag="h")
                            nc.scalar.activation(
                                h, pu, mybir.ActivationFunctionType.Gelu,
                                bias=b1_sb[:, e, m:m + 1])
                            hs = bpool.tile([HD, NT], F32R, tag="hs")
                            nc.vector.tensor_mul(hs, h, pgb)
                            for mp in range(3):
                                nc.tensor.matmul(
                                    pd[mp], w2_sb[e][:, m, mp * HD:(mp + 1) * HD],
                                    hs, start=(e == 0 and m == 0), stop=False)
                    for mp in range(3):
                        nc.tensor.matmul(pd[mp], b2r_sb[:, mp * HD:(mp + 1) * HD],
                                         gates_r, start=False, stop=True)
                    dsb = []
                    for mp in range(3):
                        dt_ = spool.tile([HD, NT], F32R, tag=f"dsb{mp}", name=f"dsb{mp}")
                        nc.vector.tensor_copy(dt_, pd[mp])
                        dsb.append(dt_)
                    for mp in range(3):
                        ppj = psB.tile([HD, NT], F32, tag="ps")
                        for kc in range(3):
                            nc.tensor.matmul(ppj,
                                             wpr_sb[:, kc, mp * HD:(mp + 1) * HD],
                                             dsb[kc],
                                             start=(kc == 0), stop=(kc == 2))
                        osb = bpool.tile([HD, NT], F32, tag="osb")
                        nc.scalar.activation(osb, ppj,
                                             mybir.ActivationFunctionType.Identity,
                                             bias=bpr_sb[:, mp:mp + 1])
                        nc.sync.dma_start(
                            out=out_cm[mp * HD:(mp + 1) * HD, t0:t0 + NT],
                            in_=osb)
    nc.compile()
    return nc


def _prep_inputs(x, w_e1, b_e1, w_e2, b_e2, w_e3, b_e3, w_e4, b_e4, w_e5, b_e5,
                 w_e6, b_e6, wg1, wg2, wg3, w_qkv, w_attn_proj, b_attn_proj,
                 wg_final, w_mlp1, b_mlp1, w_mlp2, b_mlp2, w_proj, b_proj):
    f = np.float32
    shared = {}
    shared["wca"] = np.ascontiguousarray(np.stack([
        w_e1.reshape(9, HD, HD), w_e3.reshape(9, HD, HD),
        w_e5.reshape(9, HD, HD)]), dtype=f)
    shared["wcb"] = np.ascontiguousarray(np.stack([
        w_e2.reshape(9, HD, HD), w_e4.reshape(9, HD, HD),
        w_e6.reshape(9, HD, HD)]), dtype=f)
    shared["bca"] = np.ascontiguousarray(np.stack([b_e1, b_e3, b_e5], axis=1), dtype=f)
    shared["bcb"] = np.ascontiguousarray(np.stack([b_e2, b_e4, b_e6], axis=1), dtype=f)
    wgs = np.stack([wg1, wg2, wg3])
    shared["wgd"] = np.ascontiguousarray(
        (wgs[:, :, 1] - wgs[:, :, 0])[:, :, None], dtype=f)
    eb3 = np.zeros((3, 384), f)
    for e in range(3):
        eb3[e, e * 128:(e + 1) * 128] = 1.0
    shared["eb3"] = eb3
    shared["onesd"] = np.ones((1, 128), f)
    shared["wqk"] = np.ascontiguousarray(w_qkv[:, :, :256], dtype=f)
    shared["wv"] = np.ascontiguousarray(w_qkv[:, :, 256:], dtype=f)
    shared["wap"] = np.ascontiguousarray(w_attn_proj, dtype=f)
    shared["bap"] = np.ascontiguousarray(b_attn_proj.T, dtype=f)
    shared["wgf"] = np.ascontiguousarray(wg_final.reshape(3, HD, 3), dtype=f)
    shared["w1"] = np.ascontiguousarray(w_mlp1.reshape(3, 3, HD, 1536), dtype=f)
    shared["b1"] = np.ascontiguousarray(
        b_mlp1.reshape(3, 12, HD).transpose(2, 0, 1), dtype=f)
    shared["w2"] = np.ascontiguousarray(w_mlp2.reshape(3, 12, HD, C), dtype=f)
    shared["b2r"] = np.ascontiguousarray(b_mlp2, dtype=f)
    shared["wpr"] = np.ascontiguousarray(w_proj.reshape(3, HD, C), dtype=f)
    shared["bpr"] = np.ascontiguousarray(b_proj.reshape(3, HD).T, dtype=f)

    in_maps = []
    for c in range(N_CORES):
        b, half = c // 2, c % 2
        r0 = half * R
        slab = np.zeros((C, RP, SP), f)
        glo, ghi = max(0, r0 - 8), min(HH, r0 + R + 8)
        plo = glo - (r0 - 8)
        slab[:, plo:plo + (ghi - glo), 8:104] = \
            np.asarray(x[b, glo:ghi], dtype=f).transpose(2, 0, 1)
        m = dict(shared)
        m["xp"] = np.ascontiguousarray(slab)
        in_maps.append(m)
    return in_maps


def kernel(**inputs):
    global _CACHED_NC
    if _CACHED_NC is None:
        _CACHED_NC = build_kernel()
    nc = _CACHED_NC
    in_maps = _prep_inputs(**{k: np.asarray(v) for k, v in inputs.items()})
    res = run_bass_kernel_spmd(nc, in_maps, core_ids=list(range(N_CORES)))
    out = np.empty((B, HH, WW, C), np.float32)
    for c in range(N_CORES):
        b, half = c // 2, c % 2
        slab = res.results[c]["out_cm"].reshape(C, R, 96)
        out[b, :, half * R:(half + 1) * R, :] = slab.transpose(2, 1, 0)
    return out


# revision 13
# speedup vs baseline: 1.3121x; 1.2726x over previous
"""Trainium2 Bass kernel for nn_MAMoE (conv-MoE -> row attention -> MLP-MoE).

Sharding: 8 cores = (batch b in 0..3) x (H-half in 0..1). All routing is
per-token; the reference's swapaxes(1,2) means attention row r produces
output column w=r, so each core independently computes the full pipeline
for its 48 attention rows and the host reassembles along W.

All large matmuls run as float32r (full-rate fp32 storage, ~1e-3 matmul
accuracy); small-N attention matmuls run fp32.
"""
import numpy as np

import concourse.bass as bass
import concourse.mybir as mybir
import concourse.tile as tile
from concourse import bacc
from concourse.bass_utils import run_bass_kernel_spmd
from concourse.masks import make_identity

F32 = mybir.dt.float32
F32R = mybir.dt.float32r

B, HH, WW, C = 4, 96, 96, 384
HD = 128
SCALE = float((HD // 3) ** -0.5)  # 42**-0.5
N_CORES = 8
R = 48            # attention rows per core
RP = 65           # padded rows (R + 2*8) + 1 slack row for shifted flat reads
SP = 128          # padded W (96 + 2*8, padded to 512B row stride)
T = R * 96        # tokens per core = 4608
NT = 512          # tokens per MLP tile
NTILES = T // NT  # 9
GROUPS = R // 4   # 12 groups of 4 rows
GN = 4 * 96       # tokens per group = 384

TAPS_A = [
    [(dr, ds) for dr in (-1, 0, 1) for ds in (-1, 0, 1)],
    [(dr, 0) for dr in range(-4, 5)],
    [(0, ds) for ds in range(-4, 5)],
]
TAPS_B = [
    [(dr, ds) for dr in (-2, 0, 2) for ds in (-2, 0, 2)],
    [(dr, 0) for dr in range(-8, 9, 2)],
    [(0, ds) for ds in range(-8, 9, 2)],
]

_CACHED_NC = None


def build_kernel():
    nc = bacc.Bacc("TRN2", target_bir_lowering=False, debug=False)

    xp = nc.dram_tensor("xp", [C, RP, SP], F32R, kind="ExternalInput").ap()
    wca = nc.dram_tensor("wca", [3, 9, HD, HD], F32R, kind="ExternalInput").ap()
    wcb = nc.dram_tensor("wcb", [3, 9, HD, HD], F32R, kind="ExternalInput").ap()
    bca = nc.dram_tensor("bca", [HD, 3], F32, kind="ExternalInput").ap()
    bcb = nc.dram_tensor("bcb", [HD, 3], F32, kind="ExternalInput").ap()
    wgd = nc.dram_tensor("wgd", [3, HD, 1], F32R, kind="ExternalInput").ap()
    eb3 = nc.dram_tensor("eb3", [3, 384], F32R, kind="ExternalInput").ap()
    onesd = nc.dram_tensor("onesd", [1, HD], F32R, kind="ExternalInput").ap()
    wqk = nc.dram_tensor("wqk", [3, HD, 256], F32R, kind="ExternalInput").ap()
    wv = nc.dram_tensor("wv", [3, HD, HD], F32R, kind="ExternalInput").ap()
    wap = nc.dram_tensor("wap", [3, HD, HD], F32R, kind="ExternalInput").ap()
    bap = nc.dram_tensor("bap", [HD, 3], F32, kind="ExternalInput").ap()
    wgf = nc.dram_tensor("wgf", [3, HD, 3], F32R, kind="ExternalInput").ap()
    w1 = nc.dram_tensor("w1", [3, 3, HD, 1536], F32R, kind="ExternalInput").ap()
    b1 = nc.dram_tensor("b1", [HD, 3, 12], F32, kind="ExternalInput").ap()
    w2 = nc.dram_tensor("w2", [3, 12, HD, C], F32R, kind="ExternalInput").ap()
    b2r = nc.dram_tensor("b2r", [3, C], F32R, kind="ExternalInput").ap()
    wpr = nc.dram_tensor("wpr", [3, HD, C], F32R, kind="ExternalInput").ap()
    bpr = nc.dram_tensor("bpr", [HD, 3], F32, kind="ExternalInput").ap()
    out_cm = nc.dram_tensor("out_cm", [C, T], F32, kind="ExternalOutput").ap()

    with tile.TileContext(nc) as tc:
        with tc.tile_pool(name="consts", bufs=1) as consts, \
             tc.tile_pool(name="persist", bufs=1) as persist:
            ones_r = consts.tile([1, HD], F32R)
            nc.sync.dma_start(out=ones_r, in_=onesd)
            ident = consts.tile([HD, HD], F32)
            make_identity(nc, ident)

            bca_sb = persist.tile([HD, 3], F32)
            nc.sync.dma_start(out=bca_sb, in_=bca)
            bcb_sb = persist.tile([HD, 3], F32)
            nc.sync.dma_start(out=bcb_sb, in_=bcb)
            bap_sb = persist.tile([HD, 3], F32)
            nc.sync.dma_start(out=bap_sb, in_=bap)

            xc_t = [persist.tile([HD, T], F32R, tag=f"xc{i}", name=f"xc{i}") for i in range(3)]

            # ---------------- Phase A: conv MoE + attention per branch ----
            with tc.tile_pool(name="xpool", bufs=2) as xpool, \
                 tc.tile_pool(name="wpoolA", bufs=2) as wpoolA, \
                 tc.tile_pool(name="gpool", bufs=2) as gpool, \
                 tc.tile_pool(name="psC", bufs=4, space="PSUM") as psC, \
                 tc.tile_pool(name="psT", bufs=4, space="PSUM") as psT:
                for i in range(3):
                    xp_sb = xpool.tile([HD, RP, SP], F32R, tag="xp")
                    nc.sync.dma_start(out=xp_sb, in_=xp[i * HD:(i + 1) * HD])
                    wca_sb = wpoolA.tile([HD, 9, HD], F32R, tag="wca")
                    nc.sync.dma_start(out=wca_sb, in_=wca[i].rearrange("a p b -> p a b"))
                    wcb_sb = wpoolA.tile([HD, 9, HD], F32R, tag="wcb")
                    nc.sync.dma_start(out=wcb_sb, in_=wcb[i].rearrange("a p b -> p a b"))
                    wgd_sb = wpoolA.tile([HD, 1], F32R, tag="wgd")
                    nc.sync.dma_start(out=wgd_sb, in_=wgd[i])
                    wqk_sb = wpoolA.tile([HD, 256], F32R, tag="wqk")
                    nc.sync.dma_start(out=wqk_sb, in_=wqk[i])
                    wv_sb = wpoolA.tile([HD, HD], F32R, tag="wv")
                    nc.sync.dma_start(out=wv_sb, in_=wv[i])
                    wap_sb = wpoolA.tile([HD, HD], F32R, tag="wap")
                    nc.sync.dma_start(out=wap_sb, in_=wap[i])

                    xpf = xp_sb.rearrange("p r s -> p (r s)")
                    for g in range(GROUPS):
                        rb = 8 + 4 * g  # xp row of first moe row in group
                        fo = rb * SP    # flat offset of group start
                        # gate logits first so ACT/DVE gate math hides
                        # under the conv matmuls
                        plg = psC.tile([1, NT], F32, tag="ps")
                        nc.tensor.matmul(plg, wgd_sb, xpf[:, fo:fo + NT],
                                         start=True, stop=True)
                        ex = gpool.tile([1, NT], F32R, tag="ex")
                        nc.scalar.activation(ex, plg,
                                             mybir.ActivationFunctionType.Exp,
                                             scale=1.0)
                        # two expert convs over full padded rows (N=512,
                        # contiguous; shifted tails land in pad columns)
                        pa = psC.tile([HD, NT], F32, tag="ps")
                        for ti, (dr, ds) in enumerate(TAPS_A[i]):
                            nc.tensor.matmul(
                                pa, wca_sb[:, ti, :],
                                xpf[:, fo + dr * SP + ds: fo + dr * SP + ds + NT],
                                start=(ti == 0), stop=(ti == 8))
                        pb = psC.tile([HD, NT], F32, tag="ps")
                        for ti, (dr, ds) in enumerate(TAPS_B[i]):
                            nc.tensor.matmul(
                                pb, wcb_sb[:, ti, :],
                                xpf[:, fo + dr * SP + ds: fo + dr * SP + ds + NT],
                                start=(ti == 0), stop=(ti == 8))
                        pgb = psC.tile([HD, NT], F32, tag="ps")
                        nc.tensor.matmul(pgb, ones_r, ex, start=True, stop=True)
                        e1b = gpool.tile([HD, NT], F32, tag="e1b")
                        nc.vector.tensor_scalar_add(e1b, pgb, 1.0)
                        g0b = gpool.tile([HD, NT], F32, tag="g0b")
                        nc.vector.reciprocal(g0b, e1b)
                        # moe = g0*(ca - cb) + cb  (bias-add fused on ACT)
                        ca = gpool.tile([HD, NT], F32, tag="ca")
                        nc.scalar.activation(ca, pa,
                                             mybir.ActivationFunctionType.Identity,
                                             bias=bca_sb[:, i:i + 1])
                        cb = gpool.tile([HD, NT], F32, tag="cb")
                        nc.scalar.activation(cb, pb,
                                             mybir.ActivationFunctionType.Identity,
                                             bias=bcb_sb[:, i:i + 1])
                        dd = gpool.tile([HD, NT], F32, tag="dd")
                        nc.vector.tensor_sub(dd, ca, cb)
                        d2 = gpool.tile([HD, NT], F32, tag="d2")
                        nc.vector.tensor_mul(d2, dd, g0b)
                        moe = gpool.tile([HD, NT], F32R, tag="moe")
                        nc.vector.tensor_add(moe, d2, cb)
                        # q, k projections (channel-major, full rows)
                        pq = psT.tile([HD, NT], F32, tag="ps")
                        nc.tensor.matmul(pq, wqk_sb[:, 0:HD], moe,
                                         start=True, stop=True)
                        q_sb = gpool.tile([HD, NT], F32, tag="q")
                        nc.scalar.copy(q_sb, pq)
                        pk = psT.tile([HD, NT], F32, tag="ps")
                        nc.tensor.matmul(pk, wqk_sb[:, HD:256], moe,
                                         start=True, stop=True)
                        k_sb = gpool.tile([HD, NT], F32, tag="k")
                        nc.scalar.copy(k_sb, pk)
                        # vT per row: [96 tok, 128 ch]
                        pvt = psT.tile([96, 4 * HD], F32, tag="ps")
                        for j in range(4):
                            nc.tensor.matmul(pvt[:, j * HD:(j + 1) * HD],
                                             moe[:, j * SP + 8: j * SP + 104],
                                             wv_sb, start=True, stop=True)
                        vt_sb = gpool.tile([96, 4 * HD], F32, tag="vt")
                        nc.vector.tensor_copy(vt_sb, pvt)
                        # scores + softmax (no max-sub: logits are tiny)
                        psc = psT.tile([96, GN], F32, tag="ps")
                        for j in range(4):
                            nc.tensor.matmul(psc[:, j * 96:(j + 1) * 96],
                                             q_sb[:, j * SP + 8: j * SP + 104],
                                             k_sb[:, j * SP + 8: j * SP + 104],
                                             start=True, stop=True)
                        probs = gpool.tile([96, GN], F32, tag="probs")
                        nc.scalar.activation(probs, psc,
                                             mybir.ActivationFunctionType.Exp,
                                             scale=SCALE)
                        zsum = gpool.tile([96, 4], F32, tag="zsum")
                        nc.vector.tensor_reduce(
                            zsum, probs.rearrange("p (j q) -> p j q", q=96),
                            axis=mybir.AxisListType.X, op=mybir.AluOpType.add)
                        rec = gpool.tile([96, 4], F32, tag="rec")
                        nc.vector.reciprocal(rec, zsum)
                        pn = gpool.tile([96, GN], F32, tag="pn")
                        for j in range(4):
                            nc.vector.tensor_scalar_mul(
                                pn[:, j * 96:(j + 1) * 96],
                                probs[:, j * 96:(j + 1) * 96],
                                rec[:, j:j + 1])
                        ppt = psT.tile([96, GN], F32, tag="ps")
                        for j in range(4):
                            nc.tensor.transpose(ppt[:, j * 96:(j + 1) * 96],
                                                pn[:, j * 96:(j + 1) * 96],
                                                ident[:96, :96])
                        pt_sb = gpool.tile([96, GN], F32, tag="pt")
                        nc.vector.tensor_copy(pt_sb, ppt)
                        po = psT.tile([HD, GN], F32, tag="ps")
                        for j in range(4):
                            nc.tensor.matmul(po[:, j * 96:(j + 1) * 96],
                                             vt_sb[:, j * HD:(j + 1) * HD],
                                             pt_sb[:, j * 96:(j + 1) * 96],
                                             start=True, stop=True)
                        og = gpool.tile([HD, GN], F32R, tag="og")
                        nc.vector.tensor_copy(og, po)
                        pap2 = psT.tile([HD, GN], F32, tag="ps")
                        nc.tensor.matmul(pap2, wap_sb, og, start=True, stop=True)
                        nc.scalar.activation(
                            xc_t[i][:, g * GN:(g + 1) * GN], pap2,
                            mybir.ActivationFunctionType.Identity,
                            bias=bap_sb[:, i:i + 1])

            # ---------------- Phase B: final MLP MoE + proj ---------------
            with tc.tile_pool(name="wpoolB", bufs=1) as wpoolB, \
                 tc.tile_pool(name="bpool", bufs=2) as bpool, \
                 tc.tile_pool(name="spool", bufs=1) as spool, \
                 tc.tile_pool(name="gpoolB", bufs=2) as gpoolB, \
                 tc.tile_pool(name="psL", bufs=3, space="PSUM") as psL, \
                 tc.tile_pool(name="psGB", bufs=1, space="PSUM") as psGB, \
                 tc.tile_pool(name="psPG", bufs=1, space="PSUM") as psPG, \
                 tc.tile_pool(name="psB", bufs=3, space="PSUM") as psB:
                b1_sb = wpoolB.tile([HD, 3, 12], F32)
                nc.sync.dma_start(out=b1_sb, in_=b1)
                b2r_sb = wpoolB.tile([3, C], F32R)
                nc.sync.dma_start(out=b2r_sb, in_=b2r)
                wgf_sb = wpoolB.tile([HD, 3, 3], F32R)
                nc.sync.dma_start(out=wgf_sb, in_=wgf.rearrange("a p b -> p a b"))
                wpr_sb = wpoolB.tile([HD, 3, C], F32R)
                nc.sync.dma_start(out=wpr_sb, in_=wpr.rearrange("a p b -> p a b"))
                bpr_sb = wpoolB.tile([HD, 3], F32)
                nc.sync.dma_start(out=bpr_sb, in_=bpr)
                eb3_sb = wpoolB.tile([3, 384], F32R)
                nc.sync.dma_start(out=eb3_sb, in_=eb3)
                w1_sb = []
                w2_sb = []
                for e in range(3):
                    t1 = wpoolB.tile([HD, 3, 1536], F32R, tag=f"w1_{e}", name=f"w1_{e}")
                    nc.sync.dma_start(out=t1, in_=w1[e].rearrange("a p b -> p a b"))
                    w1_sb.append(t1)
                    t2 = wpoolB.tile([HD, 12, C], F32R, tag=f"w2_{e}", name=f"w2_{e}")
                    nc.sync.dma_start(out=t2, in_=w2[e].rearrange("a p b -> p a b"))
                    w2_sb.append(t2)

                def gating_part1(t):
                    """logits + token-major top-2 softmax math for tile t."""
                    t0 = t * NT
                    plg = psGB.tile([3, NT], F32, tag="ps", name="plg")
                    for kc in range(3):
                        nc.tensor.matmul(plg, wgf_sb[:, kc, :],
                                         xc_t[kc][:, t0:t0 + NT],
                                         start=(kc == 0), stop=(kc == 2))
                    lsb = gpoolB.tile([3, NT], F32, tag="lsb", name="lsb")
                    nc.vector.tensor_copy(lsb, plg)
                    plt = psGB.tile([HD, 12], F32, tag="ps", name="plt")
                    for t4 in range(4):
                        nc.tensor.transpose(plt[:, t4 * 3:(t4 + 1) * 3],
                                            lsb[:, t4 * HD:(t4 + 1) * HD],
                                            ident[:3, :3])
                    lt = gpoolB.tile([HD, 12], F32, tag="lt", name="lt")
                    nc.vector.tensor_copy(lt, plt)
                    e_sb = gpoolB.tile([HD, 12], F32, tag="e_sb", name="e_sb")
                    nc.scalar.activation(e_sb, lt,
                                         mybir.ActivationFunctionType.Exp)
                    e3 = e_sb.rearrange("p (j e) -> p j e", e=3)
                    s4 = gpoolB.tile([HD, 4], F32, tag="s4", name="s4")
                    nc.vector.tensor_reduce(s4, e3, axis=mybir.AxisListType.X,
                                            op=mybir.AluOpType.add)
                    mn = gpoolB.tile([HD, 4], F32, tag="mn", name="mn")
                    nc.vector.tensor_reduce(mn, e3, axis=mybir.AxisListType.X,
                                            op=mybir.AluOpType.min)
                    den = gpoolB.tile([HD, 4], F32, tag="den", name="den")
                    nc.vector.tensor_sub(den, s4, mn)
                    rec = gpoolB.tile([HD, 4], F32, tag="recb", name="recb")
                    nc.vector.reciprocal(rec, den)
                    gfin = gpoolB.tile([HD, 12], F32, tag="gfin", name="gfin")
                    nmask = gpoolB.tile([HD, 12], F32, tag="nmask", name="nmask")
                    for t4 in range(4):
                        sl = slice(t4 * 3, (t4 + 1) * 3)
                        nc.vector.tensor_scalar_mul(gfin[:, sl], e_sb[:, sl],
                                                    rec[:, t4:t4 + 1])
                        nc.vector.tensor_scalar(nmask[:, sl], e_sb[:, sl],
                                                mn[:, t4:t4 + 1], None,
                                                op0=mybir.AluOpType.not_equal)
                    gm = gpoolB.tile([HD, 12], F32, tag="gm", name="gm")
                    nc.vector.tensor_mul(gm, gfin, nmask)
                    return gm

                def gating_part2(gm):
                    """expert-major gates [3, NT] from token-major gm."""
                    pgt = psGB.tile([3, NT], F32, tag="ps", name="pgt")
                    for t4 in range(4):
                        nc.tensor.transpose(pgt[:, t4 * HD:(t4 + 1) * HD],
                                            gm[:, t4 * 3:(t4 + 1) * 3],
                                            ident)
                    gates_r = gpoolB.tile([3, NT], F32R, tag="gates",
                                          name="gates_r")
                    nc.scalar.copy(gates_r, pgt)
                    return gates_r

                gm_next = gating_part1(0)
                for t in range(NTILES):
                    t0 = t * NT
                    gates_r = gating_part2(gm_next)
                    if t + 1 < NTILES:
                        gm_next = gating_part1(t + 1)

                    pd = [psL.tile([HD, NT], F32, tag="down", name=f"pd{_i}") for _i in range(3)]
                    for e in range(3):
                        pgb = psPG.tile([HD, NT], F32, tag="pgb", name="pgb")
                        nc.tensor.matmul(pgb, eb3_sb[:, e * HD:(e + 1) * HD],
                                         gates_r, start=True, stop=True)
                        for m in range(12):
                            pu = psB.tile([HD, NT], F32, tag="ps", name="pu")
                            for kc in range(3):
                                nc.tensor.matmul(
                                    pu, w1_sb[e][:, kc, m * HD:(m + 1) * HD],
                                    xc_t[kc][:, t0:t0 + NT],
                                    start=(kc == 0), stop=(kc == 2))
                            h = bpool.tile([HD, NT], F32, tag="h")
                            nc.scalar.activation(
                                h, pu, mybir.ActivationFunctionType.Gelu,
                                bias=b1_sb[:, e, m:m + 1])
                            hs = bpool.tile([HD, NT], F32R, tag="hs")
                            nc.vector.tensor_mul(hs, h, pgb)
                            for mp in range(3):
                                nc.tensor.matmul(
                                    pd[mp], w2_sb[e][:, m, mp * HD:(mp + 1) * HD],
                                    hs, start=(e == 0 and m == 0), stop=False)
                    for mp in range(3):
                        nc.tensor.matmul(pd[mp], b2r_sb[:, mp * HD:(mp + 1) * HD],
                                         gates_r, start=False, stop=True)
                    dsb = []
                    for mp in range(3):
                        dt_ = spool.tile([HD, NT], F32R, tag=f"dsb{mp}", name=f"dsb{mp}")
                        nc.vector.tensor_copy(dt_, pd[mp])
                        dsb.append(dt_)
                    for mp in range(3):
                        ppj = psB.tile([HD, NT], F32, tag="ps", name="ppj")
                        for kc in range(3):
                            nc.tensor.matmul(ppj,
                                             wpr_sb[:, kc, mp * HD:(mp + 1) * HD],
                                             dsb[kc],
                                             start=(kc == 0), stop=(kc == 2))
                        osb = bpool.tile([HD, NT], F32, tag="osb")
                        nc.scalar.activation(osb, ppj,
                                             mybir.ActivationFunctionType.Identity,
                                             bias=bpr_sb[:, mp:mp + 1])
                        nc.sync.dma_start(
                            out=out_cm[mp * HD:(mp + 1) * HD, t0:t0 + NT],
                            in_=osb)
    nc.compile()
    return nc


def _prep_inputs(x, w_e1, b_e1, w_e2, b_e2, w_e3, b_e3, w_e4, b_e4, w_e5, b_e5,
                 w_e6, b_e6, wg1, wg2, wg3, w_qkv, w_attn_proj, b_attn_proj,
                 wg_final, w_mlp1, b_mlp1, w_mlp2, b_mlp2, w_proj, b_proj):
    f = np.float32
    shared = {}
    shared["wca"] = np.ascontiguousarray(np.stack([
        w_e1.reshape(9, HD, HD), w_e3.reshape(9, HD, HD),
        w_e5.reshape(9, HD, HD)]), dtype=f)
    shared["wcb"] = np.ascontiguousarray(np.stack([
        w_e2.reshape(9, HD, HD), w_e4.reshape(9, HD, HD),
        w_e6.reshape(9, HD, HD)]), dtype=f)
    shared["bca"] = np.ascontiguousarray(np.stack([b_e1, b_e3, b_e5], axis=1), dtype=f)
    shared["bcb"] = np.ascontiguousarray(np.stack([b_e2, b_e4, b_e6], axis=1), dtype=f)
    wgs = np.stack([wg1, wg2, wg3])
    shared["wgd"] = np.ascontiguousarray(
        (wgs[:, :, 1] - wgs[:, :, 0])[:, :, None], dtype=f)
    eb3 = np.zeros((3, 384), f)
    for e in range(3):
        eb3[e, e * 128:(e + 1) * 128] = 1.0
    shared["eb3"] = eb3
    shared["onesd"] = np.ones((1, 128), f)
    shared["wqk"] = np.ascontiguousarray(w_qkv[:, :, :256], dtype=f)
    shared["wv"] = np.ascontiguousarray(w_qkv[:, :, 256:], dtype=f)
    shared["wap"] = np.ascontiguousarray(w_attn_proj, dtype=f)
    shared["bap"] = np.ascontiguousarray(b_attn_proj.T, dtype=f)
    shared["wgf"] = np.ascontiguousarray(wg_final.reshape(3, HD, 3), dtype=f)
    shared["w1"] = np.ascontiguousarray(w_mlp1.reshape(3, 3, HD, 1536), dtype=f)
    shared["b1"] = np.ascontiguousarray(
        b_mlp1.reshape(3, 12, HD).transpose(2, 0, 1), dtype=f)
    shared["w2"] = np.ascontiguousarray(w_mlp2.reshape(3, 12, HD, C), dtype=f)
    shared["b2r"] = np.ascontiguousarray(b_mlp2, dtype=f)
    shared["wpr"] = np.ascontiguousarray(w_proj.reshape(3, HD, C), dtype=f)
    shared["bpr"] = np.ascontiguousarray(b_proj.reshape(3, HD).T, dtype=f)

    in_maps = []
    for c in range(N_CORES):
        b, half = c // 2, c % 2
        r0 = half * R
        slab = np.zeros((C, RP, SP), f)
        glo, ghi = max(0, r0 - 8), min(HH, r0 + R + 8)
        plo = glo - (r0 - 8)
        slab[:, plo:plo + (ghi - glo), 8:104] = \
            np.asarray(x[b, glo:ghi], dtype=f).transpose(2, 0, 1)
        m = dict(shared)
        m["xp"] = np.ascontiguousarray(slab)
        in_maps.append(m)
    return in_maps


def kernel(**inputs):
    global _CACHED_NC
    if _CACHED_NC is None:
        _CACHED_NC = build_kernel()
    nc = _CACHED_NC
    in_maps = _prep_inputs(**{k: np.asarray(v) for k, v in inputs.items()})
    res = run_bass_kernel_spmd(nc, in_maps, core_ids=list(range(N_CORES)))
    out = np.empty((B, HH, WW, C), np.float32)
    for c in range(N_CORES):
        b, half = c // 2, c % 2
        slab = res.results[c]["out_cm"].reshape(C, R, 96)
        out[b, :, half * R:(half + 1) * R, :] = slab.transpose(2, 1, 0)
    return out


# revision 15
# speedup vs baseline: 1.4767x; 1.1254x over previous
"""Trainium2 Bass kernel for nn_MAMoE (conv-MoE -> row attention -> MLP-MoE).

Sharding: 8 cores = (batch b in 0..3) x (H-half in 0..1). All routing is
per-token; the reference's swapaxes(1,2) means attention row r produces
output column w=r, so each core independently computes the full pipeline
for its 48 attention rows and the host reassembles along W.

All large matmuls run as float32r (full-rate fp32 storage, ~1e-3 matmul
accuracy); small-N attention matmuls run fp32.
"""
import numpy as np

import concourse.bass as bass
import concourse.mybir as mybir
import concourse.tile as tile
from concourse import bacc
from concourse.bass_utils import run_bass_kernel_spmd
from concourse.masks import make_identity

F32 = mybir.dt.float32
F32R = mybir.dt.float32r

B, HH, WW, C = 4, 96, 96, 384
HD = 128
SCALE = float((HD // 3) ** -0.5)  # 42**-0.5
N_CORES = 8
R = 48            # attention rows per core
RP = 65           # padded rows (R + 2*8) + 1 slack row for shifted flat reads
SP = 128          # padded W (96 + 2*8, padded to 512B row stride)
T = R * 96        # tokens per core = 4608
NT = 512          # tokens per MLP tile
NTILES = T // NT  # 9
GROUPS = R // 4   # 12 groups of 4 rows
GN = 4 * 96       # tokens per group = 384

TAPS_A = [
    [(dr, ds) for dr in (-1, 0, 1) for ds in (-1, 0, 1)],
    [(dr, 0) for dr in range(-4, 5)],
    [(0, ds) for ds in range(-4, 5)],
]
TAPS_B = [
    [(dr, ds) for dr in (-2, 0, 2) for ds in (-2, 0, 2)],
    [(dr, 0) for dr in range(-8, 9, 2)],
    [(0, ds) for ds in range(-8, 9, 2)],
]

_CACHED_NC = None


def build_kernel():
    nc = bacc.Bacc("TRN2", target_bir_lowering=False, debug=False)

    xp = nc.dram_tensor("xp", [C, RP, SP], F32R, kind="ExternalInput").ap()
    wca = nc.dram_tensor("wca", [3, 9, HD, HD], F32R, kind="ExternalInput").ap()
    wcb = nc.dram_tensor("wcb", [3, 9, HD, HD], F32R, kind="ExternalInput").ap()
    bca = nc.dram_tensor("bca", [HD, 3], F32, kind="ExternalInput").ap()
    bcb = nc.dram_tensor("bcb", [HD, 3], F32, kind="ExternalInput").ap()
    wgd = nc.dram_tensor("wgd", [3, HD, 1], F32R, kind="ExternalInput").ap()
    eb3 = nc.dram_tensor("eb3", [3, 384], F32R, kind="ExternalInput").ap()
    onesd = nc.dram_tensor("onesd", [1, HD], F32R, kind="ExternalInput").ap()
    wqk = nc.dram_tensor("wqk", [3, HD, 256], F32R, kind="ExternalInput").ap()
    wv = nc.dram_tensor("wv", [3, HD, HD], F32R, kind="ExternalInput").ap()
    wap = nc.dram_tensor("wap", [3, HD, HD], F32R, kind="ExternalInput").ap()
    bap = nc.dram_tensor("bap", [HD, 3], F32, kind="ExternalInput").ap()
    wgf = nc.dram_tensor("wgf", [3, HD, 3], F32R, kind="ExternalInput").ap()
    w1 = nc.dram_tensor("w1", [3, 3, HD, 1536], F32R, kind="ExternalInput").ap()
    b1 = nc.dram_tensor("b1", [HD, 3, 12], F32, kind="ExternalInput").ap()
    w2 = nc.dram_tensor("w2", [3, 12, HD, C], F32R, kind="ExternalInput").ap()
    b2r = nc.dram_tensor("b2r", [3, C], F32R, kind="ExternalInput").ap()
    wpr = nc.dram_tensor("wpr", [3, HD, C], F32R, kind="ExternalInput").ap()
    bpr = nc.dram_tensor("bpr", [HD, 3], F32, kind="ExternalInput").ap()
    out_cm = nc.dram_tensor("out_cm", [C, T], F32, kind="ExternalOutput").ap()

    with tile.TileContext(nc) as tc:
        with tc.tile_pool(name="consts", bufs=1) as consts, \
             tc.tile_pool(name="persist", bufs=1) as persist:
            ones_r = consts.tile([1, HD], F32R)
            nc.sync.dma_start(out=ones_r, in_=onesd)
            ident = consts.tile([HD, HD], F32)
            make_identity(nc, ident)
            identb = consts.tile([HD, HD], mybir.dt.bfloat16)
            nc.vector.tensor_copy(identb, ident)

            bca_sb = persist.tile([HD, 3], F32)
            nc.sync.dma_start(out=bca_sb, in_=bca)
            bcb_sb = persist.tile([HD, 3], F32)
            nc.sync.dma_start(out=bcb_sb, in_=bcb)
            bap_sb = persist.tile([HD, 3], F32)
            nc.sync.dma_start(out=bap_sb, in_=bap)

            xc_t = [persist.tile([HD, T], F32R, tag=f"xc{i}", name=f"xc{i}") for i in range(3)]

            # ---------------- Phase A: conv MoE + attention per branch ----
            with tc.tile_pool(name="xpool", bufs=2) as xpool, \
                 tc.tile_pool(name="wpoolA", bufs=2) as wpoolA, \
                 tc.tile_pool(name="gpool", bufs=2) as gpool, \
                 tc.tile_pool(name="psC", bufs=4, space="PSUM") as psC, \
                 tc.tile_pool(name="psT", bufs=4, space="PSUM") as psT:
                for i in range(3):
                    xp_sb = xpool.tile([HD, RP, SP], F32R, tag="xp")
                    nc.sync.dma_start(out=xp_sb, in_=xp[i * HD:(i + 1) * HD])
                    wca_sb = wpoolA.tile([HD, 9, HD], F32R, tag="wca")
                    nc.sync.dma_start(out=wca_sb, in_=wca[i].rearrange("a p b -> p a b"))
                    wcb_sb = wpoolA.tile([HD, 9, HD], F32R, tag="wcb")
                    nc.sync.dma_start(out=wcb_sb, in_=wcb[i].rearrange("a p b -> p a b"))
                    wgd_sb = wpoolA.tile([HD, 1], F32R, tag="wgd")
                    nc.sync.dma_start(out=wgd_sb, in_=wgd[i])
                    wqk_sb = wpoolA.tile([HD, 256], F32R, tag="wqk")
                    nc.sync.dma_start(out=wqk_sb, in_=wqk[i])
                    wv_sb = wpoolA.tile([HD, HD], F32R, tag="wv")
                    nc.sync.dma_start(out=wv_sb, in_=wv[i])
                    wap_sb = wpoolA.tile([HD, HD], F32R, tag="wap")
                    nc.sync.dma_start(out=wap_sb, in_=wap[i])

                    xpf = xp_sb.rearrange("p r s -> p (r s)")
                    for g in range(GROUPS):
                        rb = 8 + 4 * g  # xp row of first moe row in group
                        fo = rb * SP    # flat offset of group start
                        # gate logits first so ACT/DVE gate math hides
                        # under the conv matmuls
                        plg = psC.tile([1, NT], F32, tag="ps")
                        nc.tensor.matmul(plg, wgd_sb, xpf[:, fo:fo + NT],
                                         start=True, stop=True)
                        ex = gpool.tile([1, NT], F32R, tag="ex")
                        nc.scalar.activation(ex, plg,
                                             mybir.ActivationFunctionType.Exp,
                                             scale=1.0)
                        # two expert convs over full padded rows (N=512,
                        # contiguous; shifted tails land in pad columns)
                        pa = psC.tile([HD, NT], F32, tag="ps")
                        for ti, (dr, ds) in enumerate(TAPS_A[i]):
                            nc.tensor.matmul(
                                pa, wca_sb[:, ti, :],
                                xpf[:, fo + dr * SP + ds: fo + dr * SP + ds + NT],
                                start=(ti == 0), stop=(ti == 8))
                        pb = psC.tile([HD, NT], F32, tag="ps")
                        for ti, (dr, ds) in enumerate(TAPS_B[i]):
                            nc.tensor.matmul(
                                pb, wcb_sb[:, ti, :],
                                xpf[:, fo + dr * SP + ds: fo + dr * SP + ds + NT],
                                start=(ti == 0), stop=(ti == 8))
                        pgb = psC.tile([HD, NT], F32, tag="ps")
                        nc.tensor.matmul(pgb, ones_r, ex, start=True, stop=True)
                        e1b = gpool.tile([HD, NT], F32, tag="e1b")
                        nc.vector.tensor_scalar_add(e1b, pgb, 1.0)
                        g0b = gpool.tile([HD, NT], F32, tag="g0b")
                        nc.vector.reciprocal(g0b, e1b)
                        # moe = g0*(ca - cb) + cb  (bias-add fused on ACT)
                        ca = gpool.tile([HD, NT], F32, tag="ca")
                        nc.scalar.activation(ca, pa,
                                             mybir.ActivationFunctionType.Identity,
                                             bias=bca_sb[:, i:i + 1])
                        cb = gpool.tile([HD, NT], F32, tag="cb")
                        nc.scalar.activation(cb, pb,
                                             mybir.ActivationFunctionType.Identity,
                                             bias=bcb_sb[:, i:i + 1])
                        dd = gpool.tile([HD, NT], F32, tag="dd")
                        nc.vector.tensor_sub(dd, ca, cb)
                        d2 = gpool.tile([HD, NT], F32, tag="d2")
                        nc.vector.tensor_mul(d2, dd, g0b)
                        moe = gpool.tile([HD, NT], F32R, tag="moe")
                        nc.vector.tensor_add(moe, d2, cb)
                        # q, k projections (channel-major, full rows)
                        pq = psT.tile([HD, NT], F32, tag="ps")
                        nc.tensor.matmul(pq, wqk_sb[:, 0:HD], moe,
                                         start=True, stop=True)
                        q_sb = gpool.tile([HD, NT], mybir.dt.bfloat16, tag="q")
                        nc.scalar.copy(q_sb, pq)
                        pk = psT.tile([HD, NT], F32, tag="ps")
                        nc.tensor.matmul(pk, wqk_sb[:, HD:256], moe,
                                         start=True, stop=True)
                        k_sb = gpool.tile([HD, NT], mybir.dt.bfloat16, tag="k")
                        nc.scalar.copy(k_sb, pk)
                        # vT per row: [96 tok, 128 ch]
                        pvt = psT.tile([96, 4 * HD], F32, tag="ps")
                        for j in range(4):
                            nc.tensor.matmul(pvt[:, j * HD:(j + 1) * HD],
                                             moe[:, j * SP + 8: j * SP + 104],
                                             wv_sb, start=True, stop=True)
                        vt_sb = gpool.tile([96, 4 * HD], mybir.dt.bfloat16, tag="vt")
                        nc.vector.tensor_copy(vt_sb, pvt)
                        # scores + softmax (no max-sub: logits are tiny)
                        psc = psT.tile([96, GN], F32, tag="ps")
                        for j in range(4):
                            nc.tensor.matmul(psc[:, j * 96:(j + 1) * 96],
                                             q_sb[:, j * SP + 8: j * SP + 104],
                                             k_sb[:, j * SP + 8: j * SP + 104],
                                             start=True, stop=True)
                        probs = gpool.tile([96, GN], mybir.dt.bfloat16, tag="probs")
                        nc.scalar.activation(probs, psc,
                                             mybir.ActivationFunctionType.Exp,
                                             scale=SCALE)
                        zsum = gpool.tile([96, 4], F32, tag="zsum")
                        nc.vector.tensor_reduce(
                            zsum, probs.rearrange("p (j q) -> p j q", q=96),
                            axis=mybir.AxisListType.X, op=mybir.AluOpType.add)
                        rec = gpool.tile([96, 4], F32, tag="rec")
                        nc.vector.reciprocal(rec, zsum)
                        pn = gpool.tile([96, GN], mybir.dt.bfloat16, tag="pn")
                        for j in range(4):
                            nc.vector.tensor_scalar_mul(
                                pn[:, j * 96:(j + 1) * 96],
                                probs[:, j * 96:(j + 1) * 96],
                                rec[:, j:j + 1])
                        ppt = psT.tile([96, GN], mybir.dt.bfloat16, tag="ps")
                        for j in range(4):
                            nc.tensor.transpose(ppt[:, j * 96:(j + 1) * 96],
                                                pn[:, j * 96:(j + 1) * 96],
                                                identb[:96, :96])
                        pt_sb = gpool.tile([96, GN], mybir.dt.bfloat16, tag="pt")
                        nc.vector.tensor_copy(pt_sb, ppt)
                        po = psT.tile([HD, GN], F32, tag="ps")
                        for j in range(4):
                            nc.tensor.matmul(po[:, j * 96:(j + 1) * 96],
                                             vt_sb[:, j * HD:(j + 1) * HD],
                                             pt_sb[:, j * 96:(j + 1) * 96],
                                             start=True, stop=True)
                        og = gpool.tile([HD, GN], F32R, tag="og")
                        nc.vector.tensor_copy(og, po)
                        pap2 = psT.tile([HD, GN], F32, tag="ps")
                        nc.tensor.matmul(pap2, wap_sb, og, start=True, stop=True)
                        nc.scalar.activation(
                            xc_t[i][:, g * GN:(g + 1) * GN], pap2,
                            mybir.ActivationFunctionType.Identity,
                            bias=bap_sb[:, i:i + 1])

            # ---------------- Phase B: final MLP MoE + proj ---------------
            with tc.tile_pool(name="wpoolB", bufs=1) as wpoolB, \
                 tc.tile_pool(name="bpool", bufs=2) as bpool, \
                 tc.tile_pool(name="spool", bufs=1) as spool, \
                 tc.tile_pool(name="gpoolB", bufs=2) as gpoolB, \
                 tc.tile_pool(name="psL", bufs=3, space="PSUM") as psL, \
                 tc.tile_pool(name="psGB", bufs=1, space="PSUM") as psGB, \
                 tc.tile_pool(name="psPG", bufs=1, space="PSUM") as psPG, \
                 tc.tile_pool(name="psB", bufs=3, space="PSUM") as psB:
                b1_sb = wpoolB.tile([HD, 3, 12], F32)
                nc.sync.dma_start(out=b1_sb, in_=b1)
                b2r_sb = wpoolB.tile([3, C], F32R)
                nc.sync.dma_start(out=b2r_sb, in_=b2r)
                wgf_sb = wpoolB.tile([HD, 3, 3], F32R)
                nc.sync.dma_start(out=wgf_sb, in_=wgf.rearrange("a p b -> p a b"))
                wpr_sb = wpoolB.tile([HD, 3, C], F32R)
                nc.sync.dma_start(out=wpr_sb, in_=wpr.rearrange("a p b -> p a b"))
                bpr_sb = wpoolB.tile([HD, 3], F32)
                nc.sync.dma_start(out=bpr_sb, in_=bpr)
                eb3_sb = wpoolB.tile([3, 384], F32R)
                nc.sync.dma_start(out=eb3_sb, in_=eb3)
                w1_sb = []
                w2_sb = []
                for e in range(3):
                    t1 = wpoolB.tile([HD, 3, 1536], F32R, tag=f"w1_{e}", name=f"w1_{e}")
                    nc.sync.dma_start(out=t1, in_=w1[e].rearrange("a p b -> p a b"))
                    w1_sb.append(t1)
                    t2 = wpoolB.tile([HD, 12, C], F32R, tag=f"w2_{e}", name=f"w2_{e}")
                    nc.sync.dma_start(out=t2, in_=w2[e].rearrange("a p b -> p a b"))
                    w2_sb.append(t2)

                def gating_part1(t):
                    """logits + token-major top-2 softmax math for tile t."""
                    t0 = t * NT
                    plg = psGB.tile([3, NT], F32, tag="ps", name="plg")
                    for kc in range(3):
                        nc.tensor.matmul(plg, wgf_sb[:, kc, :],
                                         xc_t[kc][:, t0:t0 + NT],
                                         start=(kc == 0), stop=(kc == 2))
                    lsb = gpoolB.tile([3, NT], F32, tag="lsb", name="lsb")
                    nc.vector.tensor_copy(lsb, plg)
                    plt = psGB.tile([HD, 12], F32, tag="ps", name="plt")
                    for t4 in range(4):
                        nc.tensor.transpose(plt[:, t4 * 3:(t4 + 1) * 3],
                                            lsb[:, t4 * HD:(t4 + 1) * HD],
                                            ident[:3, :3])
                    lt = gpoolB.tile([HD, 12], F32, tag="lt", name="lt")
                    nc.vector.tensor_copy(lt, plt)
                    e_sb = gpoolB.tile([HD, 12], F32, tag="e_sb", name="e_sb")
                    nc.scalar.activation(e_sb, lt,
                                         mybir.ActivationFunctionType.Exp)
                    e3 = e_sb.rearrange("p (j e) -> p j e", e=3)
                    s4 = gpoolB.tile([HD, 4], F32, tag="s4", name="s4")
                    nc.vector.tensor_reduce(s4, e3, axis=mybir.AxisListType.X,
                                            op=mybir.AluOpType.add)
                    mn = gpoolB.tile([HD, 4], F32, tag="mn", name="mn")
                    nc.vector.tensor_reduce(mn, e3, axis=mybir.AxisListType.X,
                                            op=mybir.AluOpType.min)
                    den = gpoolB.tile([HD, 4], F32, tag="den", name="den")
                    nc.vector.tensor_sub(den, s4, mn)
                    rec = gpoolB.tile([HD, 4], F32, tag="recb", name="recb")
                    nc.vector.reciprocal(rec, den)
                    gfin = gpoolB.tile([HD, 12], F32, tag="gfin", name="gfin")
                    nmask = gpoolB.tile([HD, 12], F32, tag="nmask", name="nmask")
                    for t4 in range(4):
                        sl = slice(t4 * 3, (t4 + 1) * 3)
                        nc.vector.tensor_scalar_mul(gfin[:, sl], e_sb[:, sl],
                                                    rec[:, t4:t4 + 1])
                        nc.vector.tensor_scalar(nmask[:, sl], e_sb[:, sl],
                                                mn[:, t4:t4 + 1], None,
                                                op0=mybir.AluOpType.not_equal)
                    gm = gpoolB.tile([HD, 12], F32, tag="gm", name="gm")
                    nc.vector.tensor_mul(gm, gfin, nmask)
                    return gm

                def gating_part2(gm):
                    """expert-major gates [3, NT] from token-major gm."""
                    pgt = psGB.tile([3, NT], F32, tag="ps", name="pgt")
                    for t4 in range(4):
                        nc.tensor.transpose(pgt[:, t4 * HD:(t4 + 1) * HD],
                                            gm[:, t4 * 3:(t4 + 1) * 3],
                                            ident)
                    gates_r = gpoolB.tile([3, NT], F32R, tag="gates",
                                          name="gates_r")
                    nc.scalar.copy(gates_r, pgt)
                    return gates_r

                gm_next = gating_part1(0)
                for t in range(NTILES):
                    t0 = t * NT
                    gates_r = gating_part2(gm_next)
                    if t + 1 < NTILES:
                        gm_next = gating_part1(t + 1)

                    pd = [psL.tile([HD, NT], F32, tag="down", name=f"pd{_i}") for _i in range(3)]
                    for e in range(3):
                        pgb = psPG.tile([HD, NT], F32, tag="pgb", name="pgb")
                        nc.tensor.matmul(pgb, eb3_sb[:, e * HD:(e + 1) * HD],
                                         gates_r, start=True, stop=True)
                        for m in range(12):
                            pu = psB.tile([HD, NT], F32, tag="ps", name="pu")
                            for kc in range(3):
                                nc.tensor.matmul(
                                    pu, w1_sb[e][:, kc, m * HD:(m + 1) * HD],
                                    xc_t[kc][:, t0:t0 + NT],
                                    start=(kc == 0), stop=(kc == 2))
                            h = bpool.tile([HD, NT], F32, tag="h")
                            nc.scalar.activation(
                                h, pu, mybir.ActivationFunctionType.Gelu,
                                bias=b1_sb[:, e, m:m + 1])
                            hs = bpool.tile([HD, NT], F32R, tag="hs")
                            nc.vector.tensor_mul(hs, h, pgb)
                            for mp in range(3):
                                nc.tensor.matmul(
                                    pd[mp], w2_sb[e][:, m, mp * HD:(mp + 1) * HD],
                                    hs, start=(e == 0 and m == 0), stop=False)
                    for mp in range(3):
                        nc.tensor.matmul(pd[mp], b2r_sb[:, mp * HD:(mp + 1) * HD],
                                         gates_r, start=False, stop=True)
                    dsb = []
                    for mp in range(3):
                        dt_ = spool.tile([HD, NT], F32R, tag=f"dsb{mp}", name=f"dsb{mp}")
                        nc.vector.tensor_copy(dt_, pd[mp])
                        dsb.append(dt_)
                    for mp in range(3):
                        ppj = psB.tile([HD, NT], F32, tag="ps", name="ppj")
                        for kc in range(3):
                            nc.tensor.matmul(ppj,
                                             wpr_sb[:, kc, mp * HD:(mp + 1) * HD],
                                             dsb[kc],
                                             start=(kc == 0), stop=(kc == 2))
                        osb = bpool.tile([HD, NT], F32, tag="osb")
                        nc.scalar.activation(osb, ppj,
                                             mybir.ActivationFunctionType.Identity,
                                             bias=bpr_sb[:, mp:mp + 1])
                        nc.sync.dma_start(
                            out=out_cm[mp * HD:(mp + 1) * HD, t0:t0 + NT],
                            in_=osb)
    nc.compile()
    return nc


def _prep_inputs(x, w_e1, b_e1, w_e2, b_e2, w_e3, b_e3, w_e4, b_e4, w_e5, b_e5,
                 w_e6, b_e6, wg1, wg2, wg3, w_qkv, w_attn_proj, b_attn_proj,
                 wg_final, w_mlp1, b_mlp1, w_mlp2, b_mlp2, w_proj, b_proj):
    f = np.float32
    shared = {}
    shared["wca"] = np.ascontiguousarray(np.stack([
        w_e1.reshape(9, HD, HD), w_e3.reshape(9, HD, HD),
        w_e5.reshape(9, HD, HD)]), dtype=f)
    shared["wcb"] = np.ascontiguousarray(np.stack([
        w_e2.reshape(9, HD, HD), w_e4.reshape(9, HD, HD),
        w_e6.reshape(9, HD, HD)]), dtype=f)
    shared["bca"] = np.ascontiguousarray(np.stack([b_e1, b_e3, b_e5], axis=1), dtype=f)
    shared["bcb"] = np.ascontiguousarray(np.stack([b_e2, b_e4, b_e6], axis=1), dtype=f)
    wgs = np.stack([wg1, wg2, wg3])
    shared["wgd"] = np.ascontiguousarray(
        (wgs[:, :, 1] - wgs[:, :, 0])[:, :, None], dtype=f)
    eb3 = np.zeros((3, 384), f)
    for e in range(3):
        eb3[e, e * 128:(e + 1) * 128] = 1.0
    shared["eb3"] = eb3
    shared["onesd"] = np.ones((1, 128), f)
    shared["wqk"] = np.ascontiguousarray(w_qkv[:, :, :256], dtype=f)
    shared["wv"] = np.ascontiguousarray(w_qkv[:, :, 256:], dtype=f)
    shared["wap"] = np.ascontiguousarray(w_attn_proj, dtype=f)
    shared["bap"] = np.ascontiguousarray(b_attn_proj.T, dtype=f)
    shared["wgf"] = np.ascontiguousarray(wg_final.reshape(3, HD, 3), dtype=f)
    shared["w1"] = np.ascontiguousarray(w_mlp1.reshape(3, 3, HD, 1536), dtype=f)
    shared["b1"] = np.ascontiguousarray(
        b_mlp1.reshape(3, 12, HD).transpose(2, 0, 1), dtype=f)
    shared["w2"] = np.ascontiguousarray(w_mlp2.reshape(3, 12, HD, C), dtype=f)
    shared["b2r"] = np.ascontiguousarray(b_mlp2, dtype=f)
    shared["wpr"] = np.ascontiguousarray(w_proj.reshape(3, HD, C), dtype=f)
    shared["bpr"] = np.ascontiguousarray(b_proj.reshape(3, HD).T, dtype=f)

    in_maps = []
    for c in range(N_CORES):
        b, half = c // 2, c % 2
        r0 = half * R
        slab = np.zeros((C, RP, SP), f)
        glo, ghi = max(0, r0 - 8), min(HH, r0 + R + 8)
        plo = glo - (r0 - 8)
        slab[:, plo:plo + (ghi - glo), 8:104] = \
            np.asarray(x[b, glo:ghi], dtype=f).transpose(2, 0, 1)
        m = dict(shared)
        m["xp"] = np.ascontiguousarray(slab)
        in_maps.append(m)
    return in_maps


def kernel(**inputs):
    global _CACHED_NC
    if _CACHED_NC is None:
        _CACHED_NC = build_kernel()
    nc = _CACHED_NC
    in_maps = _prep_inputs(**{k: np.asarray(v) for k, v in inputs.items()})
    res = run_bass_kernel_spmd(nc, in_maps, core_ids=list(range(N_CORES)))
    out = np.empty((B, HH, WW, C), np.float32)
    for c in range(N_CORES):
        b, half = c // 2, c % 2
        slab = res.results[c]["out_cm"].reshape(C, R, 96)
        out[b, :, half * R:(half + 1) * R, :] = slab.transpose(2, 1, 0)
    return out


# revision 16
# speedup vs baseline: 1.5478x; 1.0481x over previous
"""Trainium2 Bass kernel for nn_MAMoE (conv-MoE -> row attention -> MLP-MoE).

Sharding: 8 cores = (batch b in 0..3) x (H-half in 0..1). All routing is
per-token; the reference's swapaxes(1,2) means attention row r produces
output column w=r, so each core independently computes the full pipeline
for its 48 attention rows and the host reassembles along W.

All large matmuls run as float32r (full-rate fp32 storage, ~1e-3 matmul
accuracy); small-N attention matmuls run fp32.
"""
import numpy as np

import concourse.bass as bass
import concourse.mybir as mybir
import concourse.tile as tile
from concourse import bacc
from concourse.bass_utils import run_bass_kernel_spmd
from concourse.masks import make_identity

F32 = mybir.dt.float32
F32R = mybir.dt.float32r

B, HH, WW, C = 4, 96, 96, 384
HD = 128
SCALE = float((HD // 3) ** -0.5)  # 42**-0.5
N_CORES = 8
R = 48            # attention rows per core
RP = 65           # padded rows (R + 2*8) + 1 slack row for shifted flat reads
SP = 128          # padded W (96 + 2*8, padded to 512B row stride)
T = R * 96        # tokens per core = 4608
NT = 512          # tokens per MLP tile
NTILES = T // NT  # 9
GROUPS = R // 4   # 12 groups of 4 rows
GN = 4 * 96       # tokens per group = 384

TAPS_A = [
    [(dr, ds) for dr in (-1, 0, 1) for ds in (-1, 0, 1)],
    [(dr, 0) for dr in range(-4, 5)],
    [(0, ds) for ds in range(-4, 5)],
]
TAPS_B = [
    [(dr, ds) for dr in (-2, 0, 2) for ds in (-2, 0, 2)],
    [(dr, 0) for dr in range(-8, 9, 2)],
    [(0, ds) for ds in range(-8, 9, 2)],
]

_CACHED_NC = None


def build_kernel():
    nc = bacc.Bacc("TRN2", target_bir_lowering=False, debug=False)

    xp = nc.dram_tensor("xp", [C, RP, SP], F32R, kind="ExternalInput").ap()
    wca = nc.dram_tensor("wca", [3, 9, HD, HD], F32R, kind="ExternalInput").ap()
    wcb = nc.dram_tensor("wcb", [3, 9, HD, HD], F32R, kind="ExternalInput").ap()
    bca = nc.dram_tensor("bca", [HD, 3], F32, kind="ExternalInput").ap()
    bcb = nc.dram_tensor("bcb", [HD, 3], F32, kind="ExternalInput").ap()
    wgd = nc.dram_tensor("wgd", [3, HD, 1], F32R, kind="ExternalInput").ap()
    eb3 = nc.dram_tensor("eb3", [3, 384], F32R, kind="ExternalInput").ap()
    onesd = nc.dram_tensor("onesd", [1, HD], F32R, kind="ExternalInput").ap()
    wqk = nc.dram_tensor("wqk", [3, HD, 256], F32R, kind="ExternalInput").ap()
    wv = nc.dram_tensor("wv", [3, HD, HD], F32R, kind="ExternalInput").ap()
    bap = nc.dram_tensor("bap", [HD, 3], F32, kind="ExternalInput").ap()
    wgf = nc.dram_tensor("wgf", [3, HD, 3], F32R, kind="ExternalInput").ap()
    w1 = nc.dram_tensor("w1", [3, 3, HD, 1536], F32R, kind="ExternalInput").ap()
    b1 = nc.dram_tensor("b1", [HD, 3, 12], F32, kind="ExternalInput").ap()
    w2 = nc.dram_tensor("w2", [3, 12, HD, C], F32R, kind="ExternalInput").ap()
    b2r = nc.dram_tensor("b2r", [3, C], F32R, kind="ExternalInput").ap()
    bpr = nc.dram_tensor("bpr", [HD, 3], F32, kind="ExternalInput").ap()
    out_cm = nc.dram_tensor("out_cm", [C, T], F32, kind="ExternalOutput").ap()

    with tile.TileContext(nc) as tc:
        with tc.tile_pool(name="consts", bufs=1) as consts, \
             tc.tile_pool(name="persist", bufs=1) as persist:
            ones_r = consts.tile([1, HD], F32R)
            nc.sync.dma_start(out=ones_r, in_=onesd)
            ident = consts.tile([HD, HD], F32)
            make_identity(nc, ident)
            identb = consts.tile([HD, HD], mybir.dt.bfloat16)
            nc.vector.tensor_copy(identb, ident)

            bca_sb = persist.tile([HD, 3], F32)
            nc.sync.dma_start(out=bca_sb, in_=bca)
            bcb_sb = persist.tile([HD, 3], F32)
            nc.sync.dma_start(out=bcb_sb, in_=bcb)
            bap_sb = persist.tile([HD, 3], F32)
            nc.sync.dma_start(out=bap_sb, in_=bap)

            xc_t = [persist.tile([HD, T], F32R, tag=f"xc{i}", name=f"xc{i}") for i in range(3)]

            # ---------------- Phase A: conv MoE + attention per branch ----
            with tc.tile_pool(name="xpool", bufs=2) as xpool, \
                 tc.tile_pool(name="wpoolA", bufs=2) as wpoolA, \
                 tc.tile_pool(name="gpool", bufs=2) as gpool, \
                 tc.tile_pool(name="psC", bufs=4, space="PSUM") as psC, \
                 tc.tile_pool(name="psT", bufs=4, space="PSUM") as psT:
                for i in range(3):
                    xp_sb = xpool.tile([HD, RP, SP], F32R, tag="xp")
                    nc.sync.dma_start(out=xp_sb, in_=xp[i * HD:(i + 1) * HD])
                    wca_sb = wpoolA.tile([HD, 9, HD], F32R, tag="wca")
                    nc.sync.dma_start(out=wca_sb, in_=wca[i].rearrange("a p b -> p a b"))
                    wcb_sb = wpoolA.tile([HD, 9, HD], F32R, tag="wcb")
                    nc.sync.dma_start(out=wcb_sb, in_=wcb[i].rearrange("a p b -> p a b"))
                    wgd_sb = wpoolA.tile([HD, 1], F32R, tag="wgd")
                    nc.sync.dma_start(out=wgd_sb, in_=wgd[i])
                    wqk_sb = wpoolA.tile([HD, 256], F32R, tag="wqk")
                    nc.sync.dma_start(out=wqk_sb, in_=wqk[i])
                    wv_sb = wpoolA.tile([HD, HD], F32R, tag="wv")
                    nc.sync.dma_start(out=wv_sb, in_=wv[i])

                    xpf = xp_sb.rearrange("p r s -> p (r s)")
                    for g in range(GROUPS):
                        rb = 8 + 4 * g  # xp row of first moe row in group
                        fo = rb * SP    # flat offset of group start
                        # gate logits first so ACT/DVE gate math hides
                        # under the conv matmuls
                        plg = psC.tile([1, NT], F32, tag="ps")
                        nc.tensor.matmul(plg, wgd_sb, xpf[:, fo:fo + NT],
                                         start=True, stop=True)
                        ex = gpool.tile([1, NT], F32R, tag="ex")
                        nc.scalar.activation(ex, plg,
                                             mybir.ActivationFunctionType.Exp,
                                             scale=1.0)
                        # two expert convs over full padded rows (N=512,
                        # contiguous; shifted tails land in pad columns)
                        pa = psC.tile([HD, NT], F32, tag="ps")
                        for ti, (dr, ds) in enumerate(TAPS_A[i]):
                            nc.tensor.matmul(
                                pa, wca_sb[:, ti, :],
                                xpf[:, fo + dr * SP + ds: fo + dr * SP + ds + NT],
                                start=(ti == 0), stop=(ti == 8))
                        pb = psC.tile([HD, NT], F32, tag="ps")
                        for ti, (dr, ds) in enumerate(TAPS_B[i]):
                            nc.tensor.matmul(
                                pb, wcb_sb[:, ti, :],
                                xpf[:, fo + dr * SP + ds: fo + dr * SP + ds + NT],
                                start=(ti == 0), stop=(ti == 8))
                        pgb = psC.tile([HD, NT], F32, tag="ps")
                        nc.tensor.matmul(pgb, ones_r, ex, start=True, stop=True)
                        e1b = gpool.tile([HD, NT], F32, tag="e1b")
                        nc.vector.tensor_scalar_add(e1b, pgb, 1.0)
                        g0b = gpool.tile([HD, NT], F32, tag="g0b")
                        nc.vector.reciprocal(g0b, e1b)
                        # moe = g0*(ca - cb) + cb  (bias-add fused on ACT)
                        ca = gpool.tile([HD, NT], F32, tag="ca")
                        nc.scalar.activation(ca, pa,
                                             mybir.ActivationFunctionType.Identity,
                                             bias=bca_sb[:, i:i + 1])
                        cb = gpool.tile([HD, NT], F32, tag="cb")
                        nc.scalar.activation(cb, pb,
                                             mybir.ActivationFunctionType.Identity,
                                             bias=bcb_sb[:, i:i + 1])
                        dd = gpool.tile([HD, NT], F32, tag="dd")
                        nc.vector.tensor_sub(dd, ca, cb)
                        d2 = gpool.tile([HD, NT], F32, tag="d2")
                        nc.vector.tensor_mul(d2, dd, g0b)
                        moe = gpool.tile([HD, NT], F32R, tag="moe")
                        nc.vector.tensor_add(moe, d2, cb)
                        # q, k projections (channel-major, full rows)
                        pq = psT.tile([HD, NT], F32, tag="ps")
                        nc.tensor.matmul(pq, wqk_sb[:, 0:HD], moe,
                                         start=True, stop=True)
                        q_sb = gpool.tile([HD, NT], mybir.dt.bfloat16, tag="q")
                        nc.scalar.copy(q_sb, pq)
                        pk = psT.tile([HD, NT], F32, tag="ps")
                        nc.tensor.matmul(pk, wqk_sb[:, HD:256], moe,
                                         start=True, stop=True)
                        k_sb = gpool.tile([HD, NT], mybir.dt.bfloat16, tag="k")
                        nc.scalar.copy(k_sb, pk)
                        # vT per row: [96 tok, 128 ch]
                        pvt = psT.tile([96, 4 * HD], F32, tag="ps")
                        for j in range(4):
                            nc.tensor.matmul(pvt[:, j * HD:(j + 1) * HD],
                                             moe[:, j * SP + 8: j * SP + 104],
                                             wv_sb, start=True, stop=True)
                        vt_sb = gpool.tile([96, 4 * HD], mybir.dt.bfloat16, tag="vt")
                        nc.vector.tensor_copy(vt_sb, pvt)
                        # scores + softmax (no max-sub: logits are tiny)
                        psc = psT.tile([96, GN], F32, tag="ps")
                        for j in range(4):
                            nc.tensor.matmul(psc[:, j * 96:(j + 1) * 96],
                                             q_sb[:, j * SP + 8: j * SP + 104],
                                             k_sb[:, j * SP + 8: j * SP + 104],
                                             start=True, stop=True)
                        probs = gpool.tile([96, GN], mybir.dt.bfloat16, tag="probs")
                        nc.scalar.activation(probs, psc,
                                             mybir.ActivationFunctionType.Exp,
                                             scale=SCALE)
                        zsum = gpool.tile([96, 4], F32, tag="zsum")
                        nc.vector.tensor_reduce(
                            zsum, probs.rearrange("p (j q) -> p j q", q=96),
                            axis=mybir.AxisListType.X, op=mybir.AluOpType.add)
                        rec = gpool.tile([96, 4], F32, tag="rec")
                        nc.vector.reciprocal(rec, zsum)
                        pn = gpool.tile([96, GN], mybir.dt.bfloat16, tag="pn")
                        for j in range(4):
                            nc.vector.tensor_scalar_mul(
                                pn[:, j * 96:(j + 1) * 96],
                                probs[:, j * 96:(j + 1) * 96],
                                rec[:, j:j + 1])
                        ppt = psT.tile([96, GN], mybir.dt.bfloat16, tag="ps")
                        for j in range(4):
                            nc.tensor.transpose(ppt[:, j * 96:(j + 1) * 96],
                                                pn[:, j * 96:(j + 1) * 96],
                                                identb[:96, :96])
                        pt_sb = gpool.tile([96, GN], mybir.dt.bfloat16, tag="pt")
                        nc.vector.tensor_copy(pt_sb, ppt)
                        po = psT.tile([HD, GN], F32, tag="ps")
                        for j in range(4):
                            nc.tensor.matmul(po[:, j * 96:(j + 1) * 96],
                                             vt_sb[:, j * HD:(j + 1) * HD],
                                             pt_sb[:, j * 96:(j + 1) * 96],
                                             start=True, stop=True)
                        nc.scalar.activation(
                            xc_t[i][:, g * GN:(g + 1) * GN], po,
                            mybir.ActivationFunctionType.Identity,
                            bias=bap_sb[:, i:i + 1])

            # ---------------- Phase B: final MLP MoE + proj ---------------
            with tc.tile_pool(name="wpoolB", bufs=1) as wpoolB, \
                 tc.tile_pool(name="bpool", bufs=2) as bpool, \
                 tc.tile_pool(name="spool", bufs=1) as spool, \
                 tc.tile_pool(name="gpoolB", bufs=2) as gpoolB, \
                 tc.tile_pool(name="psL", bufs=3, space="PSUM") as psL, \
                 tc.tile_pool(name="psGB", bufs=1, space="PSUM") as psGB, \
                 tc.tile_pool(name="psPG", bufs=1, space="PSUM") as psPG, \
                 tc.tile_pool(name="psB", bufs=3, space="PSUM") as psB:
                b1_sb = wpoolB.tile([HD, 3, 12], F32)
                nc.sync.dma_start(out=b1_sb, in_=b1)
                b2r_sb = wpoolB.tile([3, C], F32R)
                nc.sync.dma_start(out=b2r_sb, in_=b2r)
                wgf_sb = wpoolB.tile([HD, 3, 3], F32R)
                nc.sync.dma_start(out=wgf_sb, in_=wgf.rearrange("a p b -> p a b"))
                bpr_sb = wpoolB.tile([HD, 3], F32)
                nc.sync.dma_start(out=bpr_sb, in_=bpr)
                eb3_sb = wpoolB.tile([3, 384], F32R)
                nc.sync.dma_start(out=eb3_sb, in_=eb3)
                w1_sb = []
                w2_sb = []
                for e in range(3):
                    t1 = wpoolB.tile([HD, 3, 1536], F32R, tag=f"w1_{e}", name=f"w1_{e}")
                    nc.sync.dma_start(out=t1, in_=w1[e].rearrange("a p b -> p a b"))
                    w1_sb.append(t1)
                    t2 = wpoolB.tile([HD, 12, C], F32R, tag=f"w2_{e}", name=f"w2_{e}")
                    nc.sync.dma_start(out=t2, in_=w2[e].rearrange("a p b -> p a b"))
                    w2_sb.append(t2)

                def gating_part1(t):
                    """logits + token-major top-2 softmax math for tile t."""
                    t0 = t * NT
                    plg = psGB.tile([3, NT], F32, tag="ps", name="plg")
                    for kc in range(3):
                        nc.tensor.matmul(plg, wgf_sb[:, kc, :],
                                         xc_t[kc][:, t0:t0 + NT],
                                         start=(kc == 0), stop=(kc == 2))
                    lsb = gpoolB.tile([3, NT], F32, tag="lsb", name="lsb")
                    nc.vector.tensor_copy(lsb, plg)
                    plt = psGB.tile([HD, 12], F32, tag="ps", name="plt")
                    for t4 in range(4):
                        nc.tensor.transpose(plt[:, t4 * 3:(t4 + 1) * 3],
                                            lsb[:, t4 * HD:(t4 + 1) * HD],
                                            ident[:3, :3])
                    lt = gpoolB.tile([HD, 12], F32, tag="lt", name="lt")
                    nc.vector.tensor_copy(lt, plt)
                    e_sb = gpoolB.tile([HD, 12], F32, tag="e_sb", name="e_sb")
                    nc.scalar.activation(e_sb, lt,
                                         mybir.ActivationFunctionType.Exp)
                    e3 = e_sb.rearrange("p (j e) -> p j e", e=3)
                    s4 = gpoolB.tile([HD, 4], F32, tag="s4", name="s4")
                    nc.vector.tensor_reduce(s4, e3, axis=mybir.AxisListType.X,
                                            op=mybir.AluOpType.add)
                    mn = gpoolB.tile([HD, 4], F32, tag="mn", name="mn")
                    nc.vector.tensor_reduce(mn, e3, axis=mybir.AxisListType.X,
                                            op=mybir.AluOpType.min)
                    den = gpoolB.tile([HD, 4], F32, tag="den", name="den")
                    nc.vector.tensor_sub(den, s4, mn)
                    rec = gpoolB.tile([HD, 4], F32, tag="recb", name="recb")
                    nc.vector.reciprocal(rec, den)
                    gfin = gpoolB.tile([HD, 12], F32, tag="gfin", name="gfin")
                    nmask = gpoolB.tile([HD, 12], F32, tag="nmask", name="nmask")
                    for t4 in range(4):
                        sl = slice(t4 * 3, (t4 + 1) * 3)
                        nc.vector.tensor_scalar_mul(gfin[:, sl], e_sb[:, sl],
                                                    rec[:, t4:t4 + 1])
                        nc.vector.tensor_scalar(nmask[:, sl], e_sb[:, sl],
                                                mn[:, t4:t4 + 1], None,
                                                op0=mybir.AluOpType.not_equal)
                    gm = gpoolB.tile([HD, 12], F32, tag="gm", name="gm")
                    nc.vector.tensor_mul(gm, gfin, nmask)
                    return gm

                def gating_part2(gm):
                    """expert-major gates [3, NT] from token-major gm."""
                    pgt = psGB.tile([3, NT], F32, tag="ps", name="pgt")
                    for t4 in range(4):
                        nc.tensor.transpose(pgt[:, t4 * HD:(t4 + 1) * HD],
                                            gm[:, t4 * 3:(t4 + 1) * 3],
                                            ident)
                    gates_r = gpoolB.tile([3, NT], F32R, tag="gates",
                                          name="gates_r")
                    nc.scalar.copy(gates_r, pgt)
                    return gates_r

                gm_next = gating_part1(0)
                for t in range(NTILES):
                    t0 = t * NT
                    gates_r = gating_part2(gm_next)
                    if t + 1 < NTILES:
                        gm_next = gating_part1(t + 1)

                    pd = [psL.tile([HD, NT], F32, tag="down", name=f"pd{_i}") for _i in range(3)]
                    for e in range(3):
                        pgb = psPG.tile([HD, NT], F32, tag="pgb", name="pgb")
                        nc.tensor.matmul(pgb, eb3_sb[:, e * HD:(e + 1) * HD],
                                         gates_r, start=True, stop=True)
                        for m in range(12):
                            pu = psB.tile([HD, NT], F32, tag="ps", name="pu")
                            for kc in range(3):
                                nc.tensor.matmul(
                                    pu, w1_sb[e][:, kc, m * HD:(m + 1) * HD],
                                    xc_t[kc][:, t0:t0 + NT],
                                    start=(kc == 0), stop=(kc == 2))
                            h = bpool.tile([HD, NT], F32, tag="h")
                            nc.scalar.activation(
                                h, pu, mybir.ActivationFunctionType.Gelu,
                                bias=b1_sb[:, e, m:m + 1])
                            hs = bpool.tile([HD, NT], F32R, tag="hs")
                            nc.vector.tensor_mul(hs, h, pgb)
                            for mp in range(3):
                                nc.tensor.matmul(
                                    pd[mp], w2_sb[e][:, m, mp * HD:(mp + 1) * HD],
                                    hs, start=(e == 0 and m == 0), stop=False)
                    for mp in range(3):
                        nc.tensor.matmul(pd[mp], b2r_sb[:, mp * HD:(mp + 1) * HD],
                                         gates_r, start=False, stop=True)
                    for mp in range(3):
                        osb = bpool.tile([HD, NT], F32, tag="osb")
                        nc.scalar.activation(osb, pd[mp],
                                             mybir.ActivationFunctionType.Identity,
                                             bias=bpr_sb[:, mp:mp + 1])
                        nc.sync.dma_start(
                            out=out_cm[mp * HD:(mp + 1) * HD, t0:t0 + NT],
                            in_=osb)
    nc.compile()
    return nc


def _prep_inputs(x, w_e1, b_e1, w_e2, b_e2, w_e3, b_e3, w_e4, b_e4, w_e5, b_e5,
                 w_e6, b_e6, wg1, wg2, wg3, w_qkv, w_attn_proj, b_attn_proj,
                 wg_final, w_mlp1, b_mlp1, w_mlp2, b_mlp2, w_proj, b_proj):
    f = np.float32
    shared = {}
    shared["wca"] = np.ascontiguousarray(np.stack([
        w_e1.reshape(9, HD, HD), w_e3.reshape(9, HD, HD),
        w_e5.reshape(9, HD, HD)]), dtype=f)
    shared["wcb"] = np.ascontiguousarray(np.stack([
        w_e2.reshape(9, HD, HD), w_e4.reshape(9, HD, HD),
        w_e6.reshape(9, HD, HD)]), dtype=f)
    shared["bca"] = np.ascontiguousarray(np.stack([b_e1, b_e3, b_e5], axis=1), dtype=f)
    shared["bcb"] = np.ascontiguousarray(np.stack([b_e2, b_e4, b_e6], axis=1), dtype=f)
    wgs = np.stack([wg1, wg2, wg3])
    shared["wgd"] = np.ascontiguousarray(
        (wgs[:, :, 1] - wgs[:, :, 0])[:, :, None], dtype=f)
    eb3 = np.zeros((3, 384), f)
    for e in range(3):
        eb3[e, e * 128:(e + 1) * 128] = 1.0
    shared["eb3"] = eb3
    shared["onesd"] = np.ones((1, 128), f)
    shared["wqk"] = np.ascontiguousarray(w_qkv[:, :, :256], dtype=f)
    wv64 = np.asarray(w_qkv[:, :, 256:], dtype=np.float64)
    wap64 = np.asarray(w_attn_proj, dtype=np.float64)
    shared["wv"] = np.ascontiguousarray(
        np.einsum("ick,iko->ico", wv64, wap64), dtype=f)
    shared["bap"] = np.ascontiguousarray(b_attn_proj.T, dtype=f)
    shared["wgf"] = np.ascontiguousarray(wg_final.reshape(3, HD, 3), dtype=f)
    shared["w1"] = np.ascontiguousarray(w_mlp1.reshape(3, 3, HD, 1536), dtype=f)
    shared["b1"] = np.ascontiguousarray(
        b_mlp1.reshape(3, 12, HD).transpose(2, 0, 1), dtype=f)
    w2p = np.asarray(w_mlp2, dtype=np.float64) @ np.asarray(w_proj, np.float64)
    shared["w2"] = np.ascontiguousarray(w2p.reshape(3, 12, HD, C), dtype=f)
    shared["b2r"] = np.ascontiguousarray(
        np.asarray(b_mlp2, np.float64) @ np.asarray(w_proj, np.float64), dtype=f)
    shared["bpr"] = np.ascontiguousarray(b_proj.reshape(3, HD).T, dtype=f)

    in_maps = []
    for c in range(N_CORES):
        b, half = c // 2, c % 2
        r0 = half * R
        slab = np.zeros((C, RP, SP), f)
        glo, ghi = max(0, r0 - 8), min(HH, r0 + R + 8)
        plo = glo - (r0 - 8)
        slab[:, plo:plo + (ghi - glo), 8:104] = \
            np.asarray(x[b, glo:ghi], dtype=f).transpose(2, 0, 1)
        m = dict(shared)
        m["xp"] = np.ascontiguousarray(slab)
        in_maps.append(m)
    return in_maps


def kernel(**inputs):
    global _CACHED_NC
    if _CACHED_NC is None:
        _CACHED_NC = build_kernel()
    nc = _CACHED_NC
    in_maps = _prep_inputs(**{k: np.asarray(v) for k, v in inputs.items()})
    res = run_bass_kernel_spmd(nc, in_maps, core_ids=list(range(N_CORES)))
    out = np.empty((B, HH, WW, C), np.float32)
    for c in range(N_CORES):
        b, half = c // 2, c % 2
        slab = res.results[c]["out_cm"].reshape(C, R, 96)
        out[b, :, half * R:(half + 1) * R, :] = slab.transpose(2, 1, 0)
    return out


# revision 17
# speedup vs baseline: 1.7313x; 1.1186x over previous
"""Trainium2 Bass kernel for nn_MAMoE (conv-MoE -> row attention -> MLP-MoE).

Sharding: 8 cores = (batch b in 0..3) x (H-half in 0..1). All routing is
per-token; the reference's swapaxes(1,2) means attention row r produces
output column w=r, so each core independently computes the full pipeline
for its 48 attention rows and the host reassembles along W.

All large matmuls run as float32r (full-rate fp32 storage, ~1e-3 matmul
accuracy); small-N attention matmuls run fp32.
"""
import numpy as np

import concourse.bass as bass
import concourse.mybir as mybir
import concourse.tile as tile
from concourse import bacc
from concourse.bass_utils import run_bass_kernel_spmd
from concourse.masks import make_identity

F32 = mybir.dt.float32
F32R = mybir.dt.float32r

B, HH, WW, C = 4, 96, 96, 384
HD = 128
SCALE = float((HD // 3) ** -0.5)  # 42**-0.5
N_CORES = 8
R = 48            # attention rows per core
RP = 65           # padded rows (R + 2*8) + 1 slack row for shifted flat reads
SP = 128          # padded W (96 + 2*8, padded to 512B row stride)
T = R * 96        # tokens per core = 4608
NT = 512          # tokens per MLP tile
NTILES = T // NT  # 9
GROUPS = R // 4   # 12 groups of 4 rows
GN = 4 * 96       # tokens per group = 384

TAPS_A = [
    [(dr, ds) for dr in (-1, 0, 1) for ds in (-1, 0, 1)],
    [(dr, 0) for dr in range(-4, 5)],
    [(0, ds) for ds in range(-4, 5)],
]
TAPS_B = [
    [(dr, ds) for dr in (-2, 0, 2) for ds in (-2, 0, 2)],
    [(dr, 0) for dr in range(-8, 9, 2)],
    [(0, ds) for ds in range(-8, 9, 2)],
]

_CACHED_NC = None


def build_kernel():
    nc = bacc.Bacc("TRN2", target_bir_lowering=False, debug=False)

    xp = nc.dram_tensor("xp", [C, RP, SP], F32R, kind="ExternalInput").ap()
    wca = nc.dram_tensor("wca", [3, 9, HD, HD], F32R, kind="ExternalInput").ap()
    wcb = nc.dram_tensor("wcb", [3, 9, HD, HD], F32R, kind="ExternalInput").ap()
    bca = nc.dram_tensor("bca", [HD, 3], F32, kind="ExternalInput").ap()
    bcb = nc.dram_tensor("bcb", [HD, 3], F32, kind="ExternalInput").ap()
    wgd = nc.dram_tensor("wgd", [3, HD, 1], F32R, kind="ExternalInput").ap()
    eb3 = nc.dram_tensor("eb3", [3, 384], F32R, kind="ExternalInput").ap()
    onesd = nc.dram_tensor("onesd", [1, HD], F32R, kind="ExternalInput").ap()
    wqk = nc.dram_tensor("wqk", [3, HD, 256], F32R, kind="ExternalInput").ap()
    wv = nc.dram_tensor("wv", [3, HD, HD], F32R, kind="ExternalInput").ap()
    bap = nc.dram_tensor("bap", [HD, 3], F32, kind="ExternalInput").ap()
    wgf = nc.dram_tensor("wgf", [3, HD, 3], F32R, kind="ExternalInput").ap()
    w1 = nc.dram_tensor("w1", [3, 3, HD, 1536], F32R, kind="ExternalInput").ap()
    b1 = nc.dram_tensor("b1", [HD, 3, 12], F32, kind="ExternalInput").ap()
    w2 = nc.dram_tensor("w2", [3, 12, HD, C], F32R, kind="ExternalInput").ap()
    b2r = nc.dram_tensor("b2r", [3, C], F32R, kind="ExternalInput").ap()
    bpr = nc.dram_tensor("bpr", [HD, 3], F32, kind="ExternalInput").ap()
    out_cm = nc.dram_tensor("out_cm", [C, T], F32, kind="ExternalOutput").ap()

    with tile.TileContext(nc) as tc:
        with tc.tile_pool(name="consts", bufs=1) as consts, \
             tc.tile_pool(name="persist", bufs=1) as persist:
            ones_r = consts.tile([1, HD], F32R)
            nc.sync.dma_start(out=ones_r, in_=onesd)
            ident = consts.tile([HD, HD], F32)
            make_identity(nc, ident)
            identb = consts.tile([HD, HD], mybir.dt.bfloat16)
            nc.vector.tensor_copy(identb, ident)

            bca_sb = persist.tile([HD, 3], F32)
            nc.sync.dma_start(out=bca_sb, in_=bca)
            bcb_sb = persist.tile([HD, 3], F32)
            nc.sync.dma_start(out=bcb_sb, in_=bcb)
            bap_sb = persist.tile([HD, 3], F32)
            nc.sync.dma_start(out=bap_sb, in_=bap)

            xc_t = [persist.tile([HD, T], F32R, tag=f"xc{i}", name=f"xc{i}") for i in range(3)]

            # ---------------- Phase A: conv MoE + attention per branch ----
            with tc.tile_pool(name="xpool", bufs=2) as xpool, \
                 tc.tile_pool(name="wpoolA", bufs=2) as wpoolA, \
                 tc.tile_pool(name="gpool", bufs=2) as gpool, \
                 tc.tile_pool(name="psC", bufs=4, space="PSUM") as psC, \
                 tc.tile_pool(name="psT", bufs=4, space="PSUM") as psT:
                for i in range(3):
                    xp_sb = xpool.tile([HD, RP, SP], F32R, tag="xp")
                    nc.sync.dma_start(out=xp_sb, in_=xp[i * HD:(i + 1) * HD])
                    wca_sb = wpoolA.tile([HD, 9, HD], F32R, tag="wca")
                    nc.sync.dma_start(out=wca_sb, in_=wca[i].rearrange("a p b -> p a b"))
                    wcb_sb = wpoolA.tile([HD, 9, HD], F32R, tag="wcb")
                    nc.sync.dma_start(out=wcb_sb, in_=wcb[i].rearrange("a p b -> p a b"))
                    wgd_sb = wpoolA.tile([HD, 1], F32R, tag="wgd")
                    nc.sync.dma_start(out=wgd_sb, in_=wgd[i])
                    wqk_sb = wpoolA.tile([HD, 256], F32R, tag="wqk")
                    nc.sync.dma_start(out=wqk_sb, in_=wqk[i])
                    wv_sb = wpoolA.tile([HD, HD], F32R, tag="wv")
                    nc.sync.dma_start(out=wv_sb, in_=wv[i])

                    xpf = xp_sb.rearrange("p r s -> p (r s)")
                    for g in range(GROUPS):
                        rb = 8 + 4 * g  # xp row of first moe row in group
                        fo = rb * SP    # flat offset of group start
                        # gate logits first so ACT/DVE gate math hides
                        # under the conv matmuls
                        plg = psC.tile([1, NT], F32, tag="ps")
                        nc.tensor.matmul(plg, wgd_sb, xpf[:, fo:fo + NT],
                                         start=True, stop=True)
                        ex = gpool.tile([1, NT], F32R, tag="ex")
                        nc.scalar.activation(ex, plg,
                                             mybir.ActivationFunctionType.Tanh,
                                             scale=-0.5)
                        # two expert convs over full padded rows (N=512,
                        # contiguous; shifted tails land in pad columns)
                        pa = psC.tile([HD, NT], F32, tag="ps")
                        for ti, (dr, ds) in enumerate(TAPS_A[i]):
                            nc.tensor.matmul(
                                pa, wca_sb[:, ti, :],
                                xpf[:, fo + dr * SP + ds: fo + dr * SP + ds + NT],
                                start=(ti == 0), stop=(ti == 8))
                        pb = psC.tile([HD, NT], F32, tag="ps")
                        for ti, (dr, ds) in enumerate(TAPS_B[i]):
                            nc.tensor.matmul(
                                pb, wcb_sb[:, ti, :],
                                xpf[:, fo + dr * SP + ds: fo + dr * SP + ds + NT],
                                start=(ti == 0), stop=(ti == 8))
                        pgb = psC.tile([HD, NT], F32, tag="ps")
                        nc.tensor.matmul(pgb, ones_r, ex, start=True, stop=True)
                        # moe = g0*(ca - cb) + cb  (bias-add fused on ACT)
                        ca = gpool.tile([HD, NT], F32, tag="ca")
                        nc.scalar.activation(ca, pa,
                                             mybir.ActivationFunctionType.Identity,
                                             bias=bca_sb[:, i:i + 1], scale=0.5)
                        cb = gpool.tile([HD, NT], F32, tag="cb")
                        nc.scalar.activation(cb, pb,
                                             mybir.ActivationFunctionType.Identity,
                                             bias=bcb_sb[:, i:i + 1], scale=0.5)
                        dd = gpool.tile([HD, NT], F32, tag="dd")
                        nc.vector.tensor_sub(dd, ca, cb)
                        d2 = gpool.tile([HD, NT], F32, tag="d2")
                        nc.vector.tensor_mul(d2, dd, pgb)
                        ss = gpool.tile([HD, NT], F32, tag="ss")
                        nc.vector.tensor_add(ss, ca, cb)
                        moe = gpool.tile([HD, NT], F32R, tag="moe")
                        nc.vector.tensor_add(moe, ss, d2)
                        # q, k projections (channel-major, full rows)
                        pq = psT.tile([HD, NT], F32, tag="ps")
                        nc.tensor.matmul(pq, wqk_sb[:, 0:HD], moe,
                                         start=True, stop=True)
                        q_sb = gpool.tile([HD, NT], mybir.dt.bfloat16, tag="q")
                        nc.scalar.copy(q_sb, pq)
                        pk = psT.tile([HD, NT], F32, tag="ps")
                        nc.tensor.matmul(pk, wqk_sb[:, HD:256], moe,
                                         start=True, stop=True)
                        k_sb = gpool.tile([HD, NT], mybir.dt.bfloat16, tag="k")
                        nc.scalar.copy(k_sb, pk)
                        # vT per row: [96 tok, 128 ch]
                        pvt = psT.tile([96, 4 * HD], F32, tag="ps")
                        for j in range(4):
                            nc.tensor.matmul(pvt[:, j * HD:(j + 1) * HD],
                                             moe[:, j * SP + 8: j * SP + 104],
                                             wv_sb, start=True, stop=True)
                        vt_sb = gpool.tile([96, 4 * HD], mybir.dt.bfloat16, tag="vt")
                        nc.vector.tensor_copy(vt_sb, pvt)
                        # scores + softmax (no max-sub: logits are tiny)
                        psc = psT.tile([96, GN], F32, tag="ps")
                        for j in range(4):
                            nc.tensor.matmul(psc[:, j * 96:(j + 1) * 96],
                                             q_sb[:, j * SP + 8: j * SP + 104],
                                             k_sb[:, j * SP + 8: j * SP + 104],
                                             start=True, stop=True)
                        probs = gpool.tile([96, GN], mybir.dt.bfloat16, tag="probs")
                        nc.scalar.activation(probs, psc,
                                             mybir.ActivationFunctionType.Exp,
                                             scale=SCALE)
                        zsum = gpool.tile([96, 4], F32, tag="zsum")
                        nc.vector.tensor_reduce(
                            zsum, probs.rearrange("p (j q) -> p j q", q=96),
                            axis=mybir.AxisListType.X, op=mybir.AluOpType.add)
                        rec = gpool.tile([96, 4], F32, tag="rec")
                        nc.vector.reciprocal(rec, zsum)
                        pn = gpool.tile([96, GN], mybir.dt.bfloat16, tag="pn")
                        for j in range(4):
                            nc.scalar.activation(
                                pn[:, j * 96:(j + 1) * 96],
                                probs[:, j * 96:(j + 1) * 96],
                                mybir.ActivationFunctionType.Copy,
                                scale=rec[:, j:j + 1])
                        ppt = psT.tile([96, GN], mybir.dt.bfloat16, tag="ps")
                        for j in range(4):
                            nc.tensor.transpose(ppt[:, j * 96:(j + 1) * 96],
                                                pn[:, j * 96:(j + 1) * 96],
                                                identb[:96, :96])
                        pt_sb = gpool.tile([96, GN], mybir.dt.bfloat16, tag="pt")
                        nc.vector.tensor_copy(pt_sb, ppt)
                        po = psT.tile([HD, GN], F32, tag="ps")
                        for j in range(4):
                            nc.tensor.matmul(po[:, j * 96:(j + 1) * 96],
                                             vt_sb[:, j * HD:(j + 1) * HD],
                                             pt_sb[:, j * 96:(j + 1) * 96],
                                             start=True, stop=True)
                        nc.scalar.activation(
                            xc_t[i][:, g * GN:(g + 1) * GN], po,
                            mybir.ActivationFunctionType.Identity,
                            bias=bap_sb[:, i:i + 1])

            # ---------------- Phase B: final MLP MoE + proj ---------------
            with tc.tile_pool(name="wpoolB", bufs=1) as wpoolB, \
                 tc.tile_pool(name="bpool", bufs=2) as bpool, \
                 tc.tile_pool(name="spool", bufs=1) as spool, \
                 tc.tile_pool(name="gpoolB", bufs=2) as gpoolB, \
                 tc.tile_pool(name="psL", bufs=3, space="PSUM") as psL, \
                 tc.tile_pool(name="psGB", bufs=1, space="PSUM") as psGB, \
                 tc.tile_pool(name="psPG", bufs=1, space="PSUM") as psPG, \
                 tc.tile_pool(name="psB", bufs=3, space="PSUM") as psB:
                b1_sb = wpoolB.tile([HD, 3, 12], F32)
                nc.sync.dma_start(out=b1_sb, in_=b1)
                b2r_sb = wpoolB.tile([3, C], F32R)
                nc.sync.dma_start(out=b2r_sb, in_=b2r)
                wgf_sb = wpoolB.tile([HD, 3, 3], F32R)
                nc.sync.dma_start(out=wgf_sb, in_=wgf.rearrange("a p b -> p a b"))
                bpr_sb = wpoolB.tile([HD, 3], F32)
                nc.sync.dma_start(out=bpr_sb, in_=bpr)
                eb3_sb = wpoolB.tile([3, 384], F32R)
                nc.sync.dma_start(out=eb3_sb, in_=eb3)
                w1_sb = []
                w2_sb = []
                for e in range(3):
                    t1 = wpoolB.tile([HD, 3, 1536], F32R, tag=f"w1_{e}", name=f"w1_{e}")
                    nc.sync.dma_start(out=t1, in_=w1[e].rearrange("a p b -> p a b"))
                    w1_sb.append(t1)
                    t2 = wpoolB.tile([HD, 12, C], F32R, tag=f"w2_{e}", name=f"w2_{e}")
                    nc.sync.dma_start(out=t2, in_=w2[e].rearrange("a p b -> p a b"))
                    w2_sb.append(t2)

                def gating_part1(t):
                    """logits + token-major top-2 softmax math for tile t."""
                    t0 = t * NT
                    plg = psGB.tile([3, NT], F32, tag="ps", name="plg")
                    for kc in range(3):
                        nc.tensor.matmul(plg, wgf_sb[:, kc, :],
                                         xc_t[kc][:, t0:t0 + NT],
                                         start=(kc == 0), stop=(kc == 2))
                    lsb = gpoolB.tile([3, NT], F32, tag="lsb", name="lsb")
                    nc.vector.tensor_copy(lsb, plg)
                    plt = psGB.tile([HD, 12], F32, tag="ps", name="plt")
                    for t4 in range(4):
                        nc.tensor.transpose(plt[:, t4 * 3:(t4 + 1) * 3],
                                            lsb[:, t4 * HD:(t4 + 1) * HD],
                                            ident[:3, :3])
                    lt = gpoolB.tile([HD, 12], F32, tag="lt", name="lt")
                    nc.vector.tensor_copy(lt, plt)
                    l3 = lt.rearrange("p (j e) -> p j e", e=3)
                    mx = gpoolB.tile([HD, 4], F32, tag="mx", name="mx")
                    nc.vector.tensor_reduce(mx, l3, axis=mybir.AxisListType.X,
                                            op=mybir.AluOpType.max)
                    mn = gpoolB.tile([HD, 4], F32, tag="mn", name="mn")
                    nc.vector.tensor_reduce(mn, l3, axis=mybir.AxisListType.X,
                                            op=mybir.AluOpType.min)
                    sm = gpoolB.tile([HD, 4], F32, tag="sm", name="sm")
                    nc.vector.tensor_reduce(sm, l3, axis=mybir.AxisListType.X,
                                            op=mybir.AluOpType.add)
                    t1 = gpoolB.tile([HD, 4], F32, tag="t1", name="t1")
                    nc.vector.tensor_sub(t1, sm, mx)
                    mid = gpoolB.tile([HD, 4], F32, tag="mid", name="mid")
                    nc.vector.tensor_sub(mid, t1, mn)
                    dm = gpoolB.tile([HD, 4], F32, tag="dm", name="dm")
                    nc.vector.tensor_sub(dm, mx, mid)
                    th = gpoolB.tile([HD, 4], F32, tag="th", name="th")
                    nc.scalar.activation(th, dm,
                                         mybir.ActivationFunctionType.Tanh,
                                         scale=0.5)
                    gmx = gpoolB.tile([HD, 4], F32, tag="gmx", name="gmx")
                    nc.vector.tensor_scalar(gmx, th, 0.5, 0.5,
                                            op0=mybir.AluOpType.mult,
                                            op1=mybir.AluOpType.add)
                    eqx = gpoolB.tile([HD, 12], F32, tag="eqx", name="eqx")
                    eqn = gpoolB.tile([HD, 12], F32, tag="eqn", name="eqn")
                    for t4 in range(4):
                        sl = slice(t4 * 3, (t4 + 1) * 3)
                        nc.vector.tensor_scalar(eqx[:, sl], lt[:, sl],
                                                mx[:, t4:t4 + 1], None,
                                                op0=mybir.AluOpType.is_equal)
                        nc.vector.tensor_scalar(eqn[:, sl], lt[:, sl],
                                                mn[:, t4:t4 + 1], None,
                                                op0=mybir.AluOpType.is_equal)
                    # u = 1 - eqx - eqn (mid indicator); g = gmx*(eqx-u) + u
                    s1 = gpoolB.tile([HD, 12], F32, tag="s1", name="s1")
                    nc.vector.tensor_add(s1, eqx, eqn)
                    u = gpoolB.tile([HD, 12], F32, tag="u", name="u")
                    nc.vector.tensor_scalar(u, s1, -1.0, 1.0,
                                            op0=mybir.AluOpType.mult,
                                            op1=mybir.AluOpType.add)
                    d0 = gpoolB.tile([HD, 12], F32, tag="d0", name="d0")
                    nc.vector.tensor_sub(d0, eqx, u)
                    p0 = gpoolB.tile([HD, 12], F32, tag="p0", name="p0")
                    for t4 in range(4):
                        sl = slice(t4 * 3, (t4 + 1) * 3)
                        nc.vector.tensor_scalar_mul(p0[:, sl], d0[:, sl],
                                                    gmx[:, t4:t4 + 1])
                    gm = gpoolB.tile([HD, 12], F32, tag="gm", name="gm")
                    nc.vector.tensor_add(gm, p0, u)
                    return gm

                def gating_part2(gm):
                    """expert-major gates [3, NT] from token-major gm."""
                    pgt = psGB.tile([3, NT], F32, tag="ps", name="pgt")
                    for t4 in range(4):
                        nc.tensor.transpose(pgt[:, t4 * HD:(t4 + 1) * HD],
                                            gm[:, t4 * 3:(t4 + 1) * 3],
                                            ident)
                    gates_r = gpoolB.tile([3, NT], F32R, tag="gates",
                                          name="gates_r")
                    nc.scalar.copy(gates_r, pgt)
                    return gates_r

                gm_next = gating_part1(0)
                for t in range(NTILES):
                    t0 = t * NT
                    gates_r = gating_part2(gm_next)
                    if t + 1 < NTILES:
                        gm_next = gating_part1(t + 1)

                    pd = [psL.tile([HD, NT], F32, tag="down", name=f"pd{_i}") for _i in range(3)]
                    for e in range(3):
                        pgb = psPG.tile([HD, NT], F32, tag="pgb", name="pgb")
                        nc.tensor.matmul(pgb, eb3_sb[:, e * HD:(e + 1) * HD],
                                         gates_r, start=True, stop=True)
                        for m in range(12):
                            pu = psB.tile([HD, NT], F32, tag="ps", name="pu")
                            for kc in range(3):
                                nc.tensor.matmul(
                                    pu, w1_sb[e][:, kc, m * HD:(m + 1) * HD],
                                    xc_t[kc][:, t0:t0 + NT],
                                    start=(kc == 0), stop=(kc == 2))
                            h = bpool.tile([HD, NT], F32, tag="h")
                            nc.scalar.activation(
                                h, pu, mybir.ActivationFunctionType.Gelu,
                                bias=b1_sb[:, e, m:m + 1])
                            hs = bpool.tile([HD, NT], F32R, tag="hs")
                            nc.vector.tensor_mul(hs, h, pgb)
                            for mp in range(3):
                                nc.tensor.matmul(
                                    pd[mp], w2_sb[e][:, m, mp * HD:(mp + 1) * HD],
                                    hs, start=(e == 0 and m == 0), stop=False)
                    for mp in range(3):
                        nc.tensor.matmul(pd[mp], b2r_sb[:, mp * HD:(mp + 1) * HD],
                                         gates_r, start=False, stop=True)
                    for mp in range(3):
                        osb = bpool.tile([HD, NT], F32, tag="osb")
                        nc.scalar.activation(osb, pd[mp],
                                             mybir.ActivationFunctionType.Identity,
                                             bias=bpr_sb[:, mp:mp + 1])
                        nc.sync.dma_start(
                            out=out_cm[mp * HD:(mp + 1) * HD, t0:t0 + NT],
                            in_=osb)
    nc.compile()
    return nc


def _prep_inputs(x, w_e1, b_e1, w_e2, b_e2, w_e3, b_e3, w_e4, b_e4, w_e5, b_e5,
                 w_e6, b_e6, wg1, wg2, wg3, w_qkv, w_attn_proj, b_attn_proj,
                 wg_final, w_mlp1, b_mlp1, w_mlp2, b_mlp2, w_proj, b_proj):
    f = np.float32
    shared = {}
    shared["wca"] = np.ascontiguousarray(np.stack([
        w_e1.reshape(9, HD, HD), w_e3.reshape(9, HD, HD),
        w_e5.reshape(9, HD, HD)]), dtype=f)
    shared["wcb"] = np.ascontiguousarray(np.stack([
        w_e2.reshape(9, HD, HD), w_e4.reshape(9, HD, HD),
        w_e6.reshape(9, HD, HD)]), dtype=f)
    shared["bca"] = np.ascontiguousarray(
        np.stack([b_e1, b_e3, b_e5], axis=1) * 0.5, dtype=f)
    shared["bcb"] = np.ascontiguousarray(
        np.stack([b_e2, b_e4, b_e6], axis=1) * 0.5, dtype=f)
    wgs = np.stack([wg1, wg2, wg3])
    shared["wgd"] = np.ascontiguousarray(
        (wgs[:, :, 1] - wgs[:, :, 0])[:, :, None], dtype=f)
    eb3 = np.zeros((3, 384), f)
    for e in range(3):
        eb3[e, e * 128:(e + 1) * 128] = 1.0
    shared["eb3"] = eb3
    shared["onesd"] = np.ones((1, 128), f)
    shared["wqk"] = np.ascontiguousarray(w_qkv[:, :, :256], dtype=f)
    wv64 = np.asarray(w_qkv[:, :, 256:], dtype=np.float64)
    wap64 = np.asarray(w_attn_proj, dtype=np.float64)
    shared["wv"] = np.ascontiguousarray(
        np.einsum("ick,iko->ico", wv64, wap64), dtype=f)
    shared["bap"] = np.ascontiguousarray(b_attn_proj.T, dtype=f)
    shared["wgf"] = np.ascontiguousarray(wg_final.reshape(3, HD, 3), dtype=f)
    shared["w1"] = np.ascontiguousarray(w_mlp1.reshape(3, 3, HD, 1536), dtype=f)
    shared["b1"] = np.ascontiguousarray(
        b_mlp1.reshape(3, 12, HD).transpose(2, 0, 1), dtype=f)
    w2p = np.asarray(w_mlp2, dtype=np.float64) @ np.asarray(w_proj, np.float64)
    shared["w2"] = np.ascontiguousarray(w2p.reshape(3, 12, HD, C), dtype=f)
    shared["b2r"] = np.ascontiguousarray(
        np.asarray(b_mlp2, np.float64) @ np.asarray(w_proj, np.float64), dtype=f)
    shared["bpr"] = np.ascontiguousarray(b_proj.reshape(3, HD).T, dtype=f)

    in_maps = []
    for c in range(N_CORES):
        b, half = c // 2, c % 2
        r0 = half * R
        slab = np.zeros((C, RP, SP), f)
        glo, ghi = max(0, r0 - 8), min(HH, r0 + R + 8)
        plo = glo - (r0 - 8)
        slab[:, plo:plo + (ghi - glo), 8:104] = \
            np.asarray(x[b, glo:ghi], dtype=f).transpose(2, 0, 1)
        m = dict(shared)
        m["xp"] = np.ascontiguousarray(slab)
        in_maps.append(m)
    return in_maps


def kernel(**inputs):
    global _CACHED_NC
    if _CACHED_NC is None:
        _CACHED_NC = build_kernel()
    nc = _CACHED_NC
    in_maps = _prep_inputs(**{k: np.asarray(v) for k, v in inputs.items()})
    res = run_bass_kernel_spmd(nc, in_maps, core_ids=list(range(N_CORES)))
    out = np.empty((B, HH, WW, C), np.float32)
    for c in range(N_CORES):
        b, half = c // 2, c % 2
        slab = res.results[c]["out_cm"].reshape(C, R, 96)
        out[b, :, half * R:(half + 1) * R, :] = slab.transpose(2, 1, 0)
    return out
